# revision 1
# baseline (speedup 1.0000x reference)
"""Trainium2 Bass kernel for the windowed 3-channel MLP (dense_mlp).

Reference computation (B=8192):
  x [B, 6144] -> view [B, 3, 2048]
  16 overlapping windows/channel (len 256, stride 119)
  h[b,c,w,:] = win @ W1[c,w] + b1[c,w]          # [B,3,16,64]
  h = mean over c                               # [B,16,64]
  g[b,grp]   = h-grp(4 windows=256) @ W2[grp] + b2   # [B,4,64]
  out        = g.reshape(B,256) @ W3 + b3       # [B,255]

Strategy: pure data parallelism over 8 cores (B/8 = 1024 rows each).
x is cast fp16, pre-transposed and chunk-packed on the HOST into the
exact feature-major SBUF layout the kernel consumes (plain contiguous
HBM->SBUF DMAs). W1 is stored fp8 e3m4 (scaled x64 into its dense
range; 1/64 folded into W2): fp8 weights halve the LDWEIGHTS stream
(4-byte FWL), and LDWEIGHTS is ~70% serialized with the matmuls on
this codegen path, so this bought ~9us. f32 PSUM accumulate; fp16
between layers. End-to-end rel err 1.35e-2 (gate 2e-2).

On-device per core:
  - 2 batch superchunks of 512 (maximizes matmul free dim = fewest
    matmul instructions; PSUM bank limit is 512 f32); each chunk's x is
    DMA'd as 3 separate k-piece tiles [(0,3),(3,8),(8,16)] — the first
    piece is exactly the 3 tiles pair 0 needs, so layer-1 starts as
    early as possible (this fill quantum was worth 12.7us: 67.2->54.5).
  - Layer 1 as banded matmuls over 128-aligned k-tiles with host-packed
    zero-padded weight blocks (channel-mean folded into PSUM accumulation,
    1/3 folded into W1).
  - Layers 2/3 stay feature-major; layer 3 uses gT as lhsT so the output
    comes out batch-major for a contiguous DMA out.
"""

import sys

sys.path.insert(0, "/opt/trn_rl_repo")

import numpy as np

import concourse.bass as bass
import concourse.mybir as mybir
import concourse.tile as tile
from concourse import bacc
from concourse.bass_utils import run_bass_kernel_spmd

P = 128
N_CORES = 8
B_FULL = 8192
B_SHARD = B_FULL // N_CORES          # 1024
CH_LEN = 2048
N_CH = 3
K_FULL = N_CH * CH_LEN               # 6144
N_WIN = 16
WIN = 256
STRIDE = 119
N_PAIR = 8                           # window pairs (2 windows x 64 = 128 feats)
KT_CH = CH_LEN // P                  # 16 k-tiles per channel
KT_ALL = K_FULL // P                 # 48
NB = 512                             # max batch chunk (matmul free dim)
CHUNKS = [512, 512]                  # batch chunk sizes (sum = B_SHARD)
assert sum(CHUNKS) == B_SHARD
assert all(nb % P == 0 for nb in CHUNKS)
# k-split of each chunk's DMA into separate piece tiles: same 3-DMA count as
# the measured-best [(0,6),(6,11),(11,16)] split, but the first piece is only
# the 3 tiles pair 0 needs, halving the pipeline-fill quantum. (A finer
# 7-piece split measured worse: 73.4us vs 67.2us — per-DMA overhead.)
# NOTE: piece COUNT is extremely sensitive: a 4-piece split [(0,3),(3,5),
# (5,8),(8,16)] measured 69.7us and a 7-piece split 73.4us vs 54.5us here —
# each extra DMA op per chunk costs ~3-7us on this path. Do not add pieces.
K_PIECES = [(0, 3), (3, 8), (8, 16)]
KP_MAX = max(t1 - t0 for t0, t1 in K_PIECES)


def _piece_of(t):
    for pi, (t0, t1) in enumerate(K_PIECES):
        if t0 <= t < t1:
            return pi, t - t0
    raise ValueError(t)
N_OUT = 255

def _pair_tiles(m):
    """k-tiles of one channel that intersect window pair m (rows 238m..238m+374)."""
    lo = (2 * STRIDE * m) // P
    hi = (2 * STRIDE * m + 2 * STRIDE + WIN - 2 - STRIDE) // P  # (238m+374)//128
    return list(range(lo, min(hi, KT_CH - 1) + 1))

# Block order for layer-1 packed weights: for m, for c, for t.
BLOCKS = [(m, c, t) for m in range(N_PAIR) for c in range(N_CH) for t in _pair_tiles(m)]
BLK_IDX = {key: i for i, key in enumerate(BLOCKS)}
N_BLK = len(BLOCKS)                  # 90


def _pack_weights(W1, b1, W2, b2, W3, b3):
    """Host-side packing of the tiny weight tensors into device layouts.

    If W1_F8: W1 is scaled by W1_SCALE and stored e3m4 (layer-1 outputs come
    out scaled; 1/W1_SCALE is folded into W2, and W1_SCALE into b1).
    """
    W1 = np.asarray(W1, dtype=np.float32)
    ki = np.arange(P)[:, None]                    # tile-local k row
    j = np.arange(P)[None, :]                     # pair-local output feature
    w_off = j // 64                               # window within pair
    n = j % 64

    w1p = np.zeros((N_BLK, P, P), dtype=np.float32)
    for i, (m, c, t) in enumerate(BLOCKS):
        w = 2 * m + w_off                         # [1,128] window index
        koff = P * t + ki - STRIDE * w            # [128,128] k within window
        mask = (koff >= 0) & (koff < WIN)
        w1p[i] = np.where(
            mask, W1[c, w, np.clip(koff, 0, WIN - 1), n] / 3.0, 0.0
        )
    # device layout: [P(ki), N_BLK * P(j)] contiguous per partition
    w1flat = np.ascontiguousarray(w1p.transpose(1, 0, 2).reshape(P, N_BLK * P))
    if W1_F8:
        import ml_dtypes
        w1sb = (w1flat * W1_SCALE).astype(ml_dtypes.float8_e3m4)
    else:
        w1sb = w1flat.astype(np.float16)

    if FOLD_W23:
        # W23_g = W2_g @ W3[64g:64g+64]  [4][256,255]; split k into 2 tiles
        # of 128 and j into halves [128,127]; layout [P, (g kt jh), <=128]
        W2f = np.asarray(W2, dtype=np.float32)
        W3f = np.asarray(W3, dtype=np.float32)
        w23 = np.stack(
            [W2f[g] @ W3f[64 * g:64 * g + 64, :] for g in range(4)]
        )                                          # [4, 256, 255]
        if W1_F8:
            w23 = w23 / W1_SCALE
        w23 = w23.reshape(4, 2, P, 255)            # [g, kt, 128, 255]
        w23p = np.zeros((4, 2, 2, P, P), dtype=np.float32)
        w23p[:, :, 0, :, :] = w23[:, :, :, :128]
        w23p[:, :, 1, :, :127] = w23[:, :, :, 128:]
        w2sb = np.ascontiguousarray(
            w23p.transpose(3, 0, 1, 2, 4).reshape(P, 16 * P)
        ).astype(np.float16)
    else:
        # W2 [4,256,64] -> pieces [g,p][128,64] -> [P, 8, 64]
        w2p = np.asarray(W2, dtype=np.float32).reshape(4, 2, P, 64)
        if W1_F8:
            w2p = w2p / W1_SCALE
        w2sb = np.ascontiguousarray(
            w2p.transpose(2, 0, 1, 3).reshape(P, 8 * 64)
        ).astype(np.float16)

    # W3 [256,255] -> [P, 2, 255]
    w3p = np.asarray(W3, dtype=np.float32).reshape(2, P, N_OUT)
    w3sb = np.ascontiguousarray(
        w3p.transpose(1, 0, 2).reshape(P, 2 * N_OUT)
    ).astype(np.float16)

    # biases (per-partition layouts)
    b1m = np.asarray(b1, dtype=np.float32).mean(axis=0)        # [16,64]
    if W1_F8:
        b1m = b1m * W1_SCALE
    b1t = np.ascontiguousarray(b1m.reshape(N_PAIR, P).T)       # [128, 8]
    b2t = np.ascontiguousarray(np.asarray(b2, dtype=np.float32).T)  # [64, 4]
    if FOLD_W23:
        # b2 flows through W3; b3 applied feature-major, packed [128, 2]
        b3e = np.asarray(b3, np.float32) + (
            np.asarray(b2, np.float32).reshape(256) @ np.asarray(W3, np.float32)
        )
        b3t = np.zeros((P, 2), np.float32)
        b3t[:, 0] = b3e[:P]
        b3t[:N_OUT - P, 1] = b3e[P:]
    else:
        b3t = np.ascontiguousarray(
            np.broadcast_to(np.asarray(b3, dtype=np.float32), (P, N_OUT))
        )                                                      # [128, 255]
    return w1sb, w2sb, w3sb, b1t, b2t, b3t


def _pack_x_shard(x16_shard):
    """[1024, 6144] fp16 -> chunk-major feature-major [128, 48*1024].

    For each batch chunk ch (nb rows starting at b0), the block
    [128 partitions, 48*nb] holds xT[k, b] with k = 128*t + p, laid out
    t-major then b within the chunk; chunks are concatenated along the
    free axis so each chunk is one fully contiguous DMA.
    """
    xT = np.ascontiguousarray(x16_shard.T)        # [6144, 1024]
    parts = []
    b0 = 0
    for nb in CHUNKS:
        blk = xT[:, b0:b0 + nb].reshape(KT_ALL, P, nb).transpose(1, 0, 2)
        parts.append(blk.reshape(P, KT_ALL * nb))
        b0 += nb
    return np.ascontiguousarray(np.concatenate(parts, axis=1))


def build_kernel(reps=1, has_bias=False, mode="full", x_f8=None):
    if x_f8 is None:
        x_f8 = X_F8
    nc = bacc.Bacc("TRN2", target_bir_lowering=False, debug=False,
                   num_devices=N_CORES)
    f16 = mybir.dt.float16
    f32 = mybir.dt.float32
    xdt = mybir.dt.float8e3 if x_f8 else f16

    wdt = mybir.dt.float8e3 if W1_F8 else f16
    x_ext = nc.declare_dram_parameter("x", [P, KT_ALL * B_SHARD], xdt, isOutput=False)
    w1_ext = nc.declare_dram_parameter("w1", [P, N_BLK * P], wdt, isOutput=False)
    w2_ext = nc.declare_dram_parameter(
        "w2", [P, (16 * P) if FOLD_W23 else (8 * 64)], f16, isOutput=False)
    w3_ext = nc.declare_dram_parameter("w3", [P, 2 * N_OUT], f16, isOutput=False)
    b1_ext = nc.declare_dram_parameter("b1t", [P, N_PAIR], f32, isOutput=False)
    b2_ext = nc.declare_dram_parameter("b2t", [64, 4], f32, isOutput=False)
    b3_ext = nc.declare_dram_parameter(
        "b3t", [P, 2] if FOLD_W23 else [P, N_OUT], f32, isOutput=False)
    out_ext = nc.declare_dram_parameter(
        "out", [N_OUT, B_SHARD] if FOLD_W23 else [B_SHARD, N_OUT], f32,
        isOutput=True)

    with tile.TileContext(nc) as tc:
        with (
            tc.tile_pool(name="wpool", bufs=1) as wpool,
            tc.tile_pool(name="xt", bufs=6) as xt_pool,
            tc.tile_pool(name="hp", bufs=10) as hp_pool,
            tc.tile_pool(name="gt", bufs=2) as gt_pool,
            tc.tile_pool(name="osb", bufs=2) as out_pool,
            tc.tile_pool(name="ps1", bufs=3, space="PSUM") as ps1_pool,
            tc.tile_pool(name="ps2", bufs=2, space="PSUM") as ps2_pool,
            tc.tile_pool(name="ps3", bufs=2, space="PSUM") as ps3_pool,
        ):
            w1sb = wpool.tile([P, N_BLK, P], wdt)
            nc.scalar.dma_start(out=w1sb[:], in_=w1_ext.rearrange("p (b j) -> p b j", j=P))
            if FOLD_W23:
                w2sb = wpool.tile([P, 16, P], f16)
                nc.scalar.dma_start(
                    out=w2sb[:], in_=w2_ext.rearrange("p (b j) -> p b j", j=P))
            else:
                w2sb = wpool.tile([P, 8, 64], f16)
                nc.scalar.dma_start(
                    out=w2sb[:], in_=w2_ext.rearrange("p (b j) -> p b j", j=64))
            w3sb = wpool.tile([P, 2, N_OUT], f16)
            nc.scalar.dma_start(out=w3sb[:], in_=w3_ext.rearrange("p (b j) -> p b j", j=N_OUT))
            b1sb = wpool.tile([P, N_PAIR], f32)
            nc.scalar.dma_start(out=b1sb[:], in_=b1_ext[:])
            b2sb = wpool.tile([64, 4], f32)
            nc.scalar.dma_start(out=b2sb[:], in_=b2_ext[:])
            b3sb = wpool.tile([P, 2] if FOLD_W23 else [P, N_OUT], f32)
            nc.scalar.dma_start(out=b3sb[:], in_=b3_ext[:])

            xt_fix = None
            if mode == "compute":
                # persistent x chunk for compute-only probe; also satisfy out
                xt_fix = wpool.tile([P, KT_ALL, NB], xdt)
                nc.sync.dma_start(
                    out=xt_fix[:, :, :],
                    in_=x_ext[:, :KT_ALL * NB].rearrange("p (t j) -> p t j", j=NB),
                )
            if mode == "dma":
                # out is never written in the loop; write something once
                nc.scalar.dma_start(out=out_ext[0:P, :], in_=b3sb[:])

            import contextlib
            loop_cm = tc.For_i(0, reps, 1) if reps > 1 else contextlib.nullcontext()
            with loop_cm:
                _kernel_body(nc, tc, locals(), has_bias, mode, xdt, xt_fix)

    nc.compile()
    return nc


def _kernel_body(nc, tc, env, has_bias, mode="full", xdt=None, xt_fix=None):
    x_ext = env["x_ext"]
    out_ext = env["out_ext"]
    w1sb, w2sb, w3sb = env["w1sb"], env["w2sb"], env["w3sb"]
    b1sb, b2sb, b3sb = env["b1sb"], env["b2sb"], env["b3sb"]
    xt_pool = env["xt_pool"]
    hp_pool, gt_pool, out_pool = env["hp_pool"], env["gt_pool"], env["out_pool"]
    ps1_pool, ps2_pool, ps3_pool = env["ps1_pool"], env["ps2_pool"], env["ps3_pool"]
    f16 = mybir.dt.float16
    f32 = mybir.dt.float32
    if xdt is None:
        xdt = f16

    b0 = 0
    for ch, nb in enumerate(CHUNKS):
        if mode == "compute":
            xtv = xt_fix[:, :, :].rearrange("p (c t) j -> p c t j", c=N_CH)
            pieces = None
        else:
            # chunk ch of pre-transposed x, one separate tile per k-piece so
            # layer-1 pairs start as soon as their piece lands
            c0 = KT_ALL * b0
            src = x_ext[:, c0:c0 + KT_ALL * nb].rearrange(
                "p (c t j) -> p c t j", c=N_CH, j=nb
            )
            pieces = []
            for (t0, t1) in K_PIECES:
                xp_t = xt_pool.tile([P, N_CH, KP_MAX, NB], xdt, name="xpt")
                xp = xp_t[:, :, :t1 - t0, :nb]
                nc.sync.dma_start(out=xp[:], in_=src[:, :, t0:t1, :])
                pieces.append(xp)
        if mode == "dma":
            b0 += nb
            continue

        def xt_rhs(c, t):
            if pieces is None:
                return xtv[:, c, t, :]
            pi, tl = _piece_of(t)
            return pieces[pi][:, c, tl, :]

        # ---- layer 1: banded matmuls per window pair ----
        hps = {}
        for m in range(N_PAIR):
            ps_t = ps1_pool.tile([P, NB], f32, name="ps1t")
            ps = ps_t[:, :nb]
            mm_list = [(c, t) for c in range(N_CH) for t in _pair_tiles(m)]
            for i, (c, t) in enumerate(mm_list):
                nc.tensor.matmul(
                    ps[:],
                    w1sb[:, BLK_IDX[(m, c, t)], :],
                    xt_rhs(c, t),
                    start=(i == 0),
                    stop=(i == len(mm_list) - 1),
                )
            hp_t = hp_pool.tile([P, NB], f16, name="hpt")
            hp = hp_t[:, :nb]
            if has_bias:
                nc.vector.tensor_scalar_add(hp[:], ps[:], b1sb[:, m:m + 1])
            else:
                nc.vector.tensor_copy(out=hp[:], in_=ps[:])
            hps[m] = hp

        if FOLD_W23:
            # ---- fused layers 2+3: out_jh = sum_(g,kt) h @ W23 ----
            for jh in range(2):
                jw = P if jh == 0 else N_OUT - P
                psO_t = ps2_pool.tile([P, NB], f32, name="psot")
                psO = psO_t[:jw, :nb]
                for i in range(8):
                    g, kt = i // 2, i % 2
                    nc.tensor.matmul(
                        psO[:],
                        w2sb[:, (g * 2 + kt) * 2 + jh, :jw],
                        hps[2 * g + kt][:],
                        start=(i == 0),
                        stop=(i == 7),
                    )
                osb_t = out_pool.tile([P, NB], f32, name="osbt")
                osb = osb_t[:jw, :nb]
                if has_bias:
                    nc.vector.tensor_scalar_add(
                        osb[:], psO[:], b3sb[:jw, jh:jh + 1])
                else:
                    nc.vector.tensor_copy(out=osb[:], in_=psO[:])
                nc.scalar.dma_start(
                    out=out_ext[jh * P:jh * P + jw, b0:b0 + nb], in_=osb[:],
                )
            b0 += nb
            continue

        # ---- layer 2: 4 groups of 4 windows ----
        gt_t = gt_pool.tile([P, 2, NB], f16, name="gtt")
        gt = gt_t[:, :, :nb]
        for g in range(4):
            ps2_t = ps2_pool.tile([64, NB], f32, name="ps2t")
            ps2 = ps2_t[:, :nb]
            for piece in range(2):
                nc.tensor.matmul(
                    ps2[:],
                    w2sb[:, 2 * g + piece, :],
                    hps[2 * g + piece][:],
                    start=(piece == 0),
                    stop=(piece == 1),
                )
            lo = 64 * (g % 2)
            if has_bias:
                nc.vector.tensor_scalar_add(
                    gt[lo:lo + 64, g // 2], ps2[:], b2sb[:, g:g + 1],
                )
            else:
                nc.vector.tensor_copy(out=gt[lo:lo + 64, g // 2], in_=ps2[:])

        # ---- layer 3: back to batch-major ----
        nj = nb // P
        osb_t = out_pool.tile([P, NB // P, N_OUT], f32, name="osbt")
        osb = osb_t[:, :nj]
        for js in range(nj):
            ps3 = ps3_pool.tile([P, N_OUT], f32)
            for piece in range(2):
                nc.tensor.matmul(
                    ps3[:],
                    gt[:, piece, js * P:(js + 1) * P],
                    w3sb[:, piece, :],
                    start=(piece == 0),
                    stop=(piece == 1),
                )
            if has_bias:
                nc.vector.tensor_tensor(
                    osb[:, js], ps3[:], b3sb[:], mybir.AluOpType.add,
                )
            else:
                nc.vector.tensor_copy(out=osb[:, js], in_=ps3[:])
        nc.scalar.dma_start(
            out=out_ext[b0:b0 + nb, :].rearrange("(j p) n -> p j n", p=P),
            in_=osb[:],
        )
        b0 += nb


_CACHED_NC = None

# Ship x as fp8 e3m4: halves HBM traffic but adds a per-matmul rhs-stream
# penalty that outweighs it (measured 72.6us vs 67.2us fp16, and rel err
# rises to 1.90e-2 vs 1.35e-2). Keep fp16 x.
X_F8 = False

# Store W1 as fp8 e3m4 (scaled by W1_SCALE into e3m4's dense range): cuts
# LDWEIGHTS stream time 2x via 4-byte FWL. 1/W1_SCALE folds into W2.
W1_F8 = True
W1_SCALE = 64.0

# Fold W3 into W2 host-side: out = sum_g h_g @ (W2_g @ W3_blk). Halves the
# layer-2/3 matmul count and removes the gt copies; output leaves the device
# feature-major [255, B_SHARD] and is transposed on the host.
# Measured worse (81.0us vs 67.2us): the feature-major out path and the
# end-of-chunk 8-MM accumulation chains cost more than the 16 saved MMs.
FOLD_W23 = False


def _prep_in_maps(x, W1, b1, W2, b2, W3, b3):
    import ml_dtypes
    xdt = ml_dtypes.float8_e3m4 if X_F8 else np.float16
    xcast = np.asarray(x, dtype=np.float32).astype(xdt)
    w1sb, w2sb, w3sb, b1t, b2t, b3t = _pack_weights(W1, b1, W2, b2, W3, b3)
    in_maps = []
    for i in range(N_CORES):
        in_maps.append({
            "x": _pack_x_shard(xcast[i * B_SHARD:(i + 1) * B_SHARD]),
            "w1": w1sb,
            "w2": w2sb,
            "w3": w3sb,
            "b1t": b1t,
            "b2t": b2t,
            "b3t": b3t,
        })
    return in_maps


_CACHED_BIAS_NC = None


def kernel(x, W1, b1, W2, b2, W3, b3):
    global _CACHED_NC, _CACHED_BIAS_NC
    has_bias = bool(
        np.any(np.asarray(b1)) or np.any(np.asarray(b2)) or np.any(np.asarray(b3))
    )
    if has_bias:
        if _CACHED_BIAS_NC is None:
            _CACHED_BIAS_NC = build_kernel(has_bias=True, x_f8=X_F8)
        nc = _CACHED_BIAS_NC
    else:
        if _CACHED_NC is None:
            _CACHED_NC = build_kernel(x_f8=X_F8)
        nc = _CACHED_NC
    in_maps = _prep_in_maps(x, W1, b1, W2, b2, W3, b3)
    last_err = None
    for attempt in range(3):
        try:
            res = run_bass_kernel_spmd(nc, in_maps, core_ids=list(range(N_CORES)))
            break
        except Exception as e:  # transient device/axon failures
            last_err = e
            if attempt == 2:
                raise
            import time as _time
            _time.sleep(20.0)
    if FOLD_W23:
        return np.concatenate(
            [res.results[i]["out"].T for i in range(N_CORES)], axis=0)
    return np.concatenate([res.results[i]["out"] for i in range(N_CORES)], axis=0)



# revision 3
# speedup vs baseline: 1.0854x; 1.0854x over previous
"""Trainium2 Bass kernel for the windowed 3-channel MLP (dense_mlp).

Reference computation (B=8192):
  x [B, 6144] -> view [B, 3, 2048]
  16 overlapping windows/channel (len 256, stride 119)
  h[b,c,w,:] = win @ W1[c,w] + b1[c,w]          # [B,3,16,64]
  h = mean over c                               # [B,16,64]
  g[b,grp]   = h-grp(4 windows=256) @ W2[grp] + b2   # [B,4,64]
  out        = g.reshape(B,256) @ W3 + b3       # [B,255]

Strategy: pure data parallelism over 8 cores (B/8 = 1024 rows each).
x is cast fp16 and host-packed t-major feature-major so each k-piece is
one fully contiguous HBM->SBUF DMA. W1 is fp8 e3m4 (scaled x64, 1/64
folded into W2) to halve the LDWEIGHTS stream. f32 PSUM accumulate;
fp16 between layers.

On-device per core, per batch chunk of 512 (PSUM free-dim limit):
  - x streamed as N_PIECE tile-column pieces into per-chunk
    double-buffered piece tiles (manual 2-set rotation across unrolled
    loop bodies -> next body's DMA overlaps this body's compute).
  - Layer 1 banded matmuls per window pair, k-tiles consumed t-major so
    pair m starts as soon as its first tile-column lands.
  - L2 group g emitted right after pair 2g+1 (fills DMA-lag stalls and
    shrinks the tail); L3 + per-chunk contiguous out DMA at chunk end.
  - PSUM->SBUF copies spread over DVE/Pool/Act so no engine serializes
    the tail.
Bench loop: UNROLL bodies per tc.For_i iteration to amortize the
all-engine barrier at the loop back-edge.
"""

import sys

sys.path.insert(0, "/opt/trn_rl_repo")

import numpy as np

import concourse.bass as bass
import concourse.mybir as mybir
import concourse.tile as tile
from concourse import bacc
from concourse.bass_utils import run_bass_kernel_spmd

P = 128
N_CORES = 8
B_FULL = 8192
B_SHARD = B_FULL // N_CORES          # 1024
CH_LEN = 2048
N_CH = 3
K_FULL = N_CH * CH_LEN               # 6144
N_WIN = 16
WIN = 256
STRIDE = 119
N_PAIR = 8                           # window pairs (2 windows x 64 = 128 feats)
KT_CH = CH_LEN // P                  # 16 k-tiles (tile-columns) per channel
KT_ALL = K_FULL // P                 # 48
NB = 512                             # max batch chunk (matmul free dim)
CHUNKS = [512, 512]                  # batch chunk sizes (sum = B_SHARD)
assert sum(CHUNKS) == B_SHARD
assert all(nb % P == 0 for nb in CHUNKS)
N_OUT = 255

# ---- tunables ----
# x k-piece split, in tile-column units (each col = 3 ch x 128 rows).
# Pair m needs cols _pair_tiles(m); pieces sized so col c lands just
# before the L1 schedule consumes it.
K_PIECES = [(0, 2), (2, 4), (4, 6), (6, 8), (8, 10), (10, 12), (12, 14),
            (14, 16)]
# queues to issue x piece DMAs from, round-robin
XQ = ["sync", "scalar"]
# unrolled bodies per For_i iteration in the bench loop
UNROLL = 4


def _piece_of(t):
    for pi, (t0, t1) in enumerate(K_PIECES):
        if t0 <= t < t1:
            return pi, t - t0
    raise ValueError(t)


def _pair_tiles(m):
    """k-tiles of one channel that intersect window pair m (rows 238m..238m+374)."""
    lo = (2 * STRIDE * m) // P
    hi = (2 * STRIDE * m + 2 * STRIDE + WIN - 2 - STRIDE) // P  # (238m+374)//128
    return list(range(lo, min(hi, KT_CH - 1) + 1))

# Block order for layer-1 packed weights: for m, for c, for t.
BLOCKS = [(m, c, t) for m in range(N_PAIR) for c in range(N_CH) for t in _pair_tiles(m)]
BLK_IDX = {key: i for i, key in enumerate(BLOCKS)}
N_BLK = len(BLOCKS)                  # 90


def _pack_weights(W1, b1, W2, b2, W3, b3):
    """Host-side packing of the tiny weight tensors into device layouts.

    W1 is scaled by W1_SCALE and stored e3m4 (layer-1 outputs come out
    scaled; 1/W1_SCALE is folded into W2, and W1_SCALE into b1).
    """
    W1 = np.asarray(W1, dtype=np.float32)
    ki = np.arange(P)[:, None]                    # tile-local k row
    j = np.arange(P)[None, :]                     # pair-local output feature
    w_off = j // 64                               # window within pair
    n = j % 64

    w1p = np.zeros((N_BLK, P, P), dtype=np.float32)
    for i, (m, c, t) in enumerate(BLOCKS):
        w = 2 * m + w_off                         # [1,128] window index
        koff = P * t + ki - STRIDE * w            # [128,128] k within window
        mask = (koff >= 0) & (koff < WIN)
        w1p[i] = np.where(
            mask, W1[c, w, np.clip(koff, 0, WIN - 1), n] / 3.0, 0.0
        )
    # device layout: [P(ki), N_BLK * P(j)] contiguous per partition
    w1flat = np.ascontiguousarray(w1p.transpose(1, 0, 2).reshape(P, N_BLK * P))
    if W1_F8:
        import ml_dtypes
        w1sb = (w1flat * W1_SCALE).astype(ml_dtypes.float8_e3m4)
    else:
        w1sb = w1flat.astype(np.float16)

    # W2 [4,256,64] -> pieces [g,p][128,64] -> [P, 8, 64]
    w2p = np.asarray(W2, dtype=np.float32).reshape(4, 2, P, 64)
    if W1_F8:
        w2p = w2p / W1_SCALE
    w2sb = np.ascontiguousarray(
        w2p.transpose(2, 0, 1, 3).reshape(P, 8 * 64)
    ).astype(np.float16)

    # W3 [256,255] -> [P, 2, 255]
    w3p = np.asarray(W3, dtype=np.float32).reshape(2, P, N_OUT)
    w3sb = np.ascontiguousarray(
        w3p.transpose(1, 0, 2).reshape(P, 2 * N_OUT)
    ).astype(np.float16)

    # biases (per-partition layouts)
    b1m = np.asarray(b1, dtype=np.float32).mean(axis=0)        # [16,64]
    if W1_F8:
        b1m = b1m * W1_SCALE
    b1t = np.ascontiguousarray(b1m.reshape(N_PAIR, P).T)       # [128, 8]
    b2t = np.ascontiguousarray(np.asarray(b2, dtype=np.float32).T)  # [64, 4]
    b3t = np.ascontiguousarray(
        np.broadcast_to(np.asarray(b3, dtype=np.float32), (P, N_OUT))
    )                                                          # [128, 255]
    return w1sb, w2sb, w3sb, b1t, b2t, b3t


def _pack_x_shard(x16_shard):
    """[1024, 6144] fp16 -> chunk-major tile-col-major [128, 48*1024].

    For each batch chunk (nb rows starting at b0), the block
    [128 partitions, 48*nb] holds xT[k, b] with k = (c*16 + t)*128 + p,
    laid out t-major then c then b within the chunk, so a k-piece
    (tile-col range) is one fully contiguous run per partition.
    """
    parts = []
    b0 = 0
    for nb in CHUNKS:
        blk = (
            x16_shard[b0:b0 + nb]                 # [nb, 6144]
            .reshape(nb, N_CH, KT_CH, P)
            .transpose(3, 2, 1, 0)                # [P, t, c, nb]
        )
        parts.append(blk.reshape(P, KT_ALL * nb))
        b0 += nb
    return np.ascontiguousarray(np.concatenate(parts, axis=1))


def build_kernel(reps=1, has_bias=False, mode="full", unroll=None):
    if unroll is None:
        unroll = 1 if reps == 1 else UNROLL
    assert reps % unroll == 0
    n_iters = reps // unroll
    nc = bacc.Bacc("TRN2", target_bir_lowering=False, debug=False,
                   num_devices=N_CORES)
    f16 = mybir.dt.float16
    f32 = mybir.dt.float32
    xdt = f16

    wdt = mybir.dt.float8e3 if W1_F8 else f16
    x_ext = nc.declare_dram_parameter("x", [P, KT_ALL * B_SHARD], xdt, isOutput=False)
    w1_ext = nc.declare_dram_parameter("w1", [P, N_BLK * P], wdt, isOutput=False)
    w2_ext = nc.declare_dram_parameter("w2", [P, 8 * 64], f16, isOutput=False)
    w3_ext = nc.declare_dram_parameter("w3", [P, 2 * N_OUT], f16, isOutput=False)
    b1_ext = nc.declare_dram_parameter("b1t", [P, N_PAIR], f32, isOutput=False)
    b2_ext = nc.declare_dram_parameter("b2t", [64, 4], f32, isOutput=False)
    b3_ext = nc.declare_dram_parameter("b3t", [P, N_OUT], f32, isOutput=False)
    out_ext = nc.declare_dram_parameter("out", [B_SHARD, N_OUT], f32,
                                        isOutput=True)

    with tile.TileContext(nc) as tc:
        with (
            tc.tile_pool(name="wpool", bufs=1) as wpool,
            tc.tile_pool(name="hp", bufs=12) as hp_pool,
            tc.tile_pool(name="gt", bufs=3) as gt_pool,
            tc.tile_pool(name="osb", bufs=3) as out_pool,
            tc.tile_pool(name="ps1", bufs=3, space="PSUM") as ps1_pool,
            tc.tile_pool(name="ps2", bufs=2, space="PSUM") as ps2_pool,
            tc.tile_pool(name="ps3", bufs=2, space="PSUM") as ps3_pool,
        ):
            w1sb = wpool.tile([P, N_BLK, P], wdt)
            nc.scalar.dma_start(out=w1sb[:], in_=w1_ext.rearrange("p (b j) -> p b j", j=P))
            w2sb = wpool.tile([P, 8, 64], f16)
            nc.scalar.dma_start(
                out=w2sb[:], in_=w2_ext.rearrange("p (b j) -> p b j", j=64))
            w3sb = wpool.tile([P, 2, N_OUT], f16)
            nc.scalar.dma_start(out=w3sb[:], in_=w3_ext.rearrange("p (b j) -> p b j", j=N_OUT))
            b1sb = wpool.tile([P, N_PAIR], f32)
            nc.scalar.dma_start(out=b1sb[:], in_=b1_ext[:])
            b2sb = wpool.tile([64, 4], f32)
            nc.scalar.dma_start(out=b2sb[:], in_=b2_ext[:])
            b3sb = wpool.tile([P, N_OUT], f32)
            nc.scalar.dma_start(out=b3sb[:], in_=b3_ext[:])

            # manual double-buffered x piece tiles: [set][piece] where
            # set = unrolled-body parity. Writes into set s wait (WAR) for
            # the previous body-with-parity-s's consumers — so the next
            # body's x DMA overlaps this body's compute.
            xsets = []
            for s in range(2):
                ptiles = []
                for pi, (t0, t1) in enumerate(K_PIECES):
                    ptiles.append(wpool.tile(
                        [P, t1 - t0, N_CH, NB], xdt, name=f"xp{s}_{pi}"))
                xsets.append(ptiles)

            xt_fix = None
            if mode == "compute":
                # persistent x chunk for compute-only probe
                xt_fix = wpool.tile([P, KT_ALL, NB], xdt)
                nc.sync.dma_start(
                    out=xt_fix[:, :, :],
                    in_=x_ext[:, :KT_ALL * NB].rearrange("p (t j) -> p t j", j=NB),
                )
            if mode == "dma":
                # out is never written in the loop; write something once
                nc.scalar.dma_start(out=out_ext[0:P, :], in_=b3sb[:])

            import contextlib
            loop_cm = tc.For_i(0, n_iters, 1) if n_iters > 1 else contextlib.nullcontext()
            with loop_cm:
                for u in range(unroll):
                    _kernel_body(nc, tc, locals(), has_bias, mode, xdt,
                                 xt_fix, xsets[u % 2])

    nc.compile()
    return nc


def _kernel_body(nc, tc, env, has_bias, mode="full", xdt=None, xt_fix=None,
                 xset=None):
    x_ext = env["x_ext"]
    out_ext = env["out_ext"]
    w1sb, w2sb, w3sb = env["w1sb"], env["w2sb"], env["w3sb"]
    b1sb, b2sb, b3sb = env["b1sb"], env["b2sb"], env["b3sb"]
    hp_pool, gt_pool, out_pool = env["hp_pool"], env["gt_pool"], env["out_pool"]
    ps1_pool, ps2_pool, ps3_pool = env["ps1_pool"], env["ps2_pool"], env["ps3_pool"]
    f16 = mybir.dt.float16
    f32 = mybir.dt.float32
    if xdt is None:
        xdt = f16
    xqs = [getattr(nc, q) for q in XQ]

    b0 = 0
    for ch, nb in enumerate(CHUNKS):
        if mode == "compute":
            xtv = xt_fix[:, :, :].rearrange("p (c t) j -> p c t j", c=N_CH)
            pieces = None
        else:
            # chunk ch of pre-transposed t-major x: each piece is one
            # fully contiguous DMA per partition
            c0 = KT_ALL * b0
            src = x_ext[:, c0:c0 + KT_ALL * nb].rearrange(
                "p (t c j) -> p t c j", c=N_CH, j=nb
            )
            pieces = []
            for pi, (t0, t1) in enumerate(K_PIECES):
                xp = xset[pi][:, :, :, :nb]
                xqs[pi % len(xqs)].dma_start(out=xp[:], in_=src[:, t0:t1, :, :])
                pieces.append(xp)
        if mode == "dma":
            b0 += nb
            continue

        def xt_rhs(c, t):
            if pieces is None:
                return xtv[:, c, t, :]
            pi, tl = _piece_of(t)
            return pieces[pi][:, tl, c, :]

        hps = {}
        gt_t = gt_pool.tile([P, 2, NB], f16, name="gtt")
        gt = gt_t[:, :, :nb]
        for m in range(N_PAIR):
            # ---- layer 1, pair m: banded matmuls, t-major so the pair
            # starts as soon as its first tile-column lands ----
            ps_t = ps1_pool.tile([P, NB], f32, name="ps1t")
            ps = ps_t[:, :nb]
            mm_list = [(c, t) for t in _pair_tiles(m) for c in range(N_CH)]
            for i, (c, t) in enumerate(mm_list):
                nc.tensor.matmul(
                    ps[:],
                    w1sb[:, BLK_IDX[(m, c, t)], :],
                    xt_rhs(c, t),
                    start=(i == 0),
                    stop=(i == len(mm_list) - 1),
                )
            hp_t = hp_pool.tile([P, NB], f16, name="hpt")
            hp = hp_t[:, :nb]
            # alternate DVE / Act for the PSUM->SBUF h copies (GPSIMD/Pool
            # cannot read PSUM)
            if has_bias:
                if m % 2 == 0:
                    nc.vector.tensor_scalar_add(hp[:], ps[:], b1sb[:, m:m + 1])
                else:
                    nc.scalar.add(hp[:], ps[:], b1sb[:, m:m + 1])
            else:
                if m % 2 == 0:
                    nc.vector.tensor_copy(out=hp[:], in_=ps[:])
                else:
                    nc.scalar.copy(out=hp[:], in_=ps[:])
            hps[m] = hp

            # ---- layer 2, group g right after its two pairs ----
            if m % 2 == 1:
                g = m // 2
                ps2_t = ps2_pool.tile([64, NB], f32, name="ps2t")
                ps2 = ps2_t[:, :nb]
                for piece in range(2):
                    nc.tensor.matmul(
                        ps2[:],
                        w2sb[:, 2 * g + piece, :],
                        hps[2 * g + piece][:],
                        start=(piece == 0),
                        stop=(piece == 1),
                    )
                lo = 64 * (g % 2)
                if has_bias:
                    nc.vector.tensor_scalar_add(
                        gt[lo:lo + 64, g // 2], ps2[:], b2sb[:, g:g + 1],
                    )
                else:
                    nc.vector.tensor_copy(out=gt[lo:lo + 64, g // 2], in_=ps2[:])

        # ---- layer 3: back to batch-major ----
        nj = nb // P
        osb_t = out_pool.tile([P, NB // P, N_OUT], f32, name="osbt")
        osb = osb_t[:, :nj]
        for js in range(nj):
            ps3 = ps3_pool.tile([P, N_OUT], f32)
            for piece in range(2):
                nc.tensor.matmul(
                    ps3[:],
                    gt[:, piece, js * P:(js + 1) * P],
                    w3sb[:, piece, :],
                    start=(piece == 0),
                    stop=(piece == 1),
                )
            # spread the 4 osb copies over Act/DVE
            if has_bias:
                if js % 2 == 0:
                    nc.scalar.add(osb[:, js], ps3[:], b3sb[:])
                else:
                    nc.vector.tensor_tensor(
                        osb[:, js], ps3[:], b3sb[:], mybir.AluOpType.add,
                    )
            else:
                if js % 2 == 0:
                    nc.scalar.copy(out=osb[:, js], in_=ps3[:])
                else:
                    nc.vector.tensor_copy(out=osb[:, js], in_=ps3[:])
        nc.scalar.dma_start(
            out=out_ext[b0:b0 + nb, :].rearrange("(j p) n -> p j n", p=P),
            in_=osb[:],
        )
        b0 += nb


_CACHED_NC = None

# Store W1 as fp8 e3m4 (scaled by W1_SCALE into e3m4's dense range): cuts
# LDWEIGHTS stream time 2x via 4-byte FWL. 1/W1_SCALE folds into W2.
W1_F8 = True
W1_SCALE = 64.0


def _prep_in_maps(x, W1, b1, W2, b2, W3, b3):
    xcast = np.asarray(x, dtype=np.float32).astype(np.float16)
    w1sb, w2sb, w3sb, b1t, b2t, b3t = _pack_weights(W1, b1, W2, b2, W3, b3)
    in_maps = []
    for i in range(N_CORES):
        in_maps.append({
            "x": _pack_x_shard(xcast[i * B_SHARD:(i + 1) * B_SHARD]),
            "w1": w1sb,
            "w2": w2sb,
            "w3": w3sb,
            "b1t": b1t,
            "b2t": b2t,
            "b3t": b3t,
        })
    return in_maps


_CACHED_BIAS_NC = None


def kernel(x, W1, b1, W2, b2, W3, b3):
    global _CACHED_NC, _CACHED_BIAS_NC
    has_bias = bool(
        np.any(np.asarray(b1)) or np.any(np.asarray(b2)) or np.any(np.asarray(b3))
    )
    if has_bias:
        if _CACHED_BIAS_NC is None:
            _CACHED_BIAS_NC = build_kernel(has_bias=True)
        nc = _CACHED_BIAS_NC
    else:
        if _CACHED_NC is None:
            _CACHED_NC = build_kernel()
        nc = _CACHED_NC
    in_maps = _prep_in_maps(x, W1, b1, W2, b2, W3, b3)
    last_err = None
    for attempt in range(3):
        try:
            res = run_bass_kernel_spmd(nc, in_maps, core_ids=list(range(N_CORES)))
            break
        except Exception as e:  # transient device/axon failures
            last_err = e
            if attempt == 2:
                raise
            import time as _time
            _time.sleep(20.0)
    return np.concatenate([res.results[i]["out"] for i in range(N_CORES)], axis=0)


# revision 7
# speedup vs baseline: 1.1738x; 1.0815x over previous
"""Trainium2 Bass kernel for the windowed 3-channel MLP (dense_mlp).

Reference computation (B=8192):
  x [B, 6144] -> view [B, 3, 2048]
  16 overlapping windows/channel (len 256, stride 119)
  h[b,c,w,:] = win @ W1[c,w] + b1[c,w]          # [B,3,16,64]
  h = mean over c                               # [B,16,64]
  g[b,grp]   = h-grp(4 windows=256) @ W2[grp] + b2   # [B,4,64]
  out        = g.reshape(B,256) @ W3 + b3       # [B,255]

Strategy: pure data parallelism over 8 cores (B/8 = 1024 rows each).
x is cast fp16 and host-packed t-major feature-major so each k-piece is
one fully contiguous HBM->SBUF DMA. W1 is fp8 e3m4 (scaled x64, 1/64
folded into W2) to halve the LDWEIGHTS stream. f32 PSUM accumulate;
fp16 between layers.

On-device per core, per batch chunk of 512 (PSUM free-dim limit):
  - x streamed as N_PIECE tile-column pieces into per-chunk
    double-buffered piece tiles (manual 2-set rotation across unrolled
    loop bodies -> next body's DMA overlaps this body's compute).
  - Layer 1 banded matmuls per window pair, k-tiles consumed t-major so
    pair m starts as soon as its first tile-column lands.
  - L2 group g emitted right after pair 2g+1 (fills DMA-lag stalls and
    shrinks the tail); L3 + per-chunk contiguous out DMA at chunk end.
  - PSUM->SBUF copies spread over DVE/Pool/Act so no engine serializes
    the tail.
Bench loop: UNROLL bodies per tc.For_i iteration to amortize the
all-engine barrier at the loop back-edge.
"""

import sys

sys.path.insert(0, "/opt/trn_rl_repo")

import numpy as np

import concourse.bass as bass
import concourse.mybir as mybir
import concourse.tile as tile
from concourse import bacc
from concourse.bass_utils import run_bass_kernel_spmd

P = 128
N_CORES = 8
B_FULL = 8192
B_SHARD = B_FULL // N_CORES          # 1024
CH_LEN = 2048
N_CH = 3
K_FULL = N_CH * CH_LEN               # 6144
N_WIN = 16
WIN = 256
STRIDE = 119
N_PAIR = 8                           # window pairs (2 windows x 64 = 128 feats)
KT_CH = CH_LEN // P                  # 16 k-tiles (tile-columns) per channel
KT_ALL = K_FULL // P                 # 48
NB = 512                             # max batch chunk (matmul free dim)
CHUNKS = [512, 512]                  # batch chunk sizes (sum = B_SHARD)
assert sum(CHUNKS) == B_SHARD
assert all(nb % P == 0 for nb in CHUNKS)
N_OUT = 255

# ---- tunables ----
# tile-columns (0..15 per channel) whose x is shipped fp8 e3m4 instead of
# fp16. DMA-bound kernel: each fp8 col saves 1/32 of x traffic at the cost
# of quantization error (~1.9e-2 end-to-end rel err at all-16 fp8,
# ~1.66e-2 at the default odd-col half split, 1.35e-2 at none).
X8_COLS = (1, 3, 5, 7, 9, 11, 13, 15)
# piece size (stream tile-cols per DMA) for the x streams
PIECE_COLS = 2
# queues to issue x piece DMAs from, round-robin
XQ = ["sync", "scalar"]
# unrolled bodies per For_i iteration in the bench loop
UNROLL = 8


def _streams():
    """(cols_16, cols_8) tile-col lists per stream."""
    s8 = sorted(X8_COLS)
    s16 = [t for t in range(KT_CH) if t not in X8_COLS]
    return s16, s8


def _stream_pieces(cols):
    n = len(cols)
    return [(i, min(i + PIECE_COLS, n)) for i in range(0, n, PIECE_COLS)]


def _pair_tiles(m):
    """k-tiles of one channel that intersect window pair m (rows 238m..238m+374)."""
    lo = (2 * STRIDE * m) // P
    hi = (2 * STRIDE * m + 2 * STRIDE + WIN - 2 - STRIDE) // P  # (238m+374)//128
    return list(range(lo, min(hi, KT_CH - 1) + 1))

# Block order for layer-1 packed weights: for m, for c, for t.
BLOCKS = [(m, c, t) for m in range(N_PAIR) for c in range(N_CH) for t in _pair_tiles(m)]
BLK_IDX = {key: i for i, key in enumerate(BLOCKS)}
N_BLK = len(BLOCKS)                  # 90


def _pack_weights(W1, b1, W2, b2, W3, b3):
    """Host-side packing of the tiny weight tensors into device layouts.

    W1 is scaled by W1_SCALE and stored e3m4 (layer-1 outputs come out
    scaled; 1/W1_SCALE is folded into W2, and W1_SCALE into b1).
    """
    W1 = np.asarray(W1, dtype=np.float32)
    ki = np.arange(P)[:, None]                    # tile-local k row
    j = np.arange(P)[None, :]                     # pair-local output feature
    w_off = j // 64                               # window within pair
    n = j % 64

    w1p = np.zeros((N_BLK, P, P), dtype=np.float32)
    for i, (m, c, t) in enumerate(BLOCKS):
        w = 2 * m + w_off                         # [1,128] window index
        koff = P * t + ki - STRIDE * w            # [128,128] k within window
        mask = (koff >= 0) & (koff < WIN)
        w1p[i] = np.where(
            mask, W1[c, w, np.clip(koff, 0, WIN - 1), n] / 3.0, 0.0
        )
    # device layout: [P(ki), N_BLK * P(j)] contiguous per partition
    w1flat = np.ascontiguousarray(w1p.transpose(1, 0, 2).reshape(P, N_BLK * P))
    if W1_F8:
        import ml_dtypes
        w1sb = (w1flat * W1_SCALE).astype(ml_dtypes.float8_e3m4)
    else:
        w1sb = w1flat.astype(np.float16)

    # W2 [4,256,64] -> pieces [g,p][128,64] -> [P, 8, 64]
    w2p = np.asarray(W2, dtype=np.float32).reshape(4, 2, P, 64)
    if W1_F8:
        w2p = w2p / W1_SCALE
    w2sb = np.ascontiguousarray(
        w2p.transpose(2, 0, 1, 3).reshape(P, 8 * 64)
    ).astype(np.float16)

    # W3 [256,255] -> [P, 2, 255]
    w3p = np.asarray(W3, dtype=np.float32).reshape(2, P, N_OUT)
    w3sb = np.ascontiguousarray(
        w3p.transpose(1, 0, 2).reshape(P, 2 * N_OUT)
    ).astype(np.float16)

    # biases (per-partition layouts)
    b1m = np.asarray(b1, dtype=np.float32).mean(axis=0)        # [16,64]
    if W1_F8:
        b1m = b1m * W1_SCALE
    b1t = np.ascontiguousarray(b1m.reshape(N_PAIR, P).T)       # [128, 8]
    b2t = np.ascontiguousarray(np.asarray(b2, dtype=np.float32).T)  # [64, 4]
    b3t = np.ascontiguousarray(
        np.broadcast_to(np.asarray(b3, dtype=np.float32), (P, N_OUT))
    )                                                          # [128, 255]
    return w1sb, w2sb, w3sb, b1t, b2t, b3t


def _pack_x_streams(x_shard):
    """[1024, 6144] f32 -> two chunk-major tile-col-major streams.

    Per chunk the block [128, len(cols)*3*nb] holds xT[k, b] laid out
    stream-col-major then channel then batch, so a k-piece (stream-col
    range) is one fully contiguous run per partition. Stream 16 carries
    cols not in X8_COLS as fp16; stream 8 carries X8_COLS as fp8 e3m4.
    """
    import ml_dtypes
    s16, s8 = _streams()
    out = {}
    for key, cols, dt in (("x16", s16, np.float16),
                          ("x8", s8, ml_dtypes.float8_e3m4)):
        if not cols:
            continue
        parts = []
        b0 = 0
        for nb in CHUNKS:
            blk = (
                x_shard[b0:b0 + nb]               # [nb, 6144]
                .reshape(nb, N_CH, KT_CH, P)
                .transpose(3, 2, 1, 0)            # [P, t, c, nb]
            )[:, cols]                            # [P, len(cols), c, nb]
            parts.append(blk.reshape(P, len(cols) * N_CH * nb))
            b0 += nb
        out[key] = np.ascontiguousarray(
            np.concatenate(parts, axis=1)).astype(dt)
    return out


def build_kernel(reps=1, has_bias=False, mode="full", unroll=None):
    if unroll is None:
        unroll = 1 if reps == 1 else UNROLL
    assert reps % unroll == 0
    n_iters = reps // unroll
    nc = bacc.Bacc("TRN2", target_bir_lowering=False, debug=False,
                   num_devices=N_CORES)
    f16 = mybir.dt.float16
    f32 = mybir.dt.float32
    f8 = mybir.dt.float8e3
    s16, s8 = _streams()

    wdt = mybir.dt.float8e3 if W1_F8 else f16
    x_exts = {}
    if s16:
        x_exts["x16"] = nc.declare_dram_parameter(
            "x16", [P, len(s16) * N_CH * B_SHARD], f16, isOutput=False)
    if s8:
        x_exts["x8"] = nc.declare_dram_parameter(
            "x8", [P, len(s8) * N_CH * B_SHARD], f8, isOutput=False)
    w1_ext = nc.declare_dram_parameter("w1", [P, N_BLK * P], wdt, isOutput=False)
    w2_ext = nc.declare_dram_parameter("w2", [P, 8 * 64], f16, isOutput=False)
    w3_ext = nc.declare_dram_parameter("w3", [P, 2 * N_OUT], f16, isOutput=False)
    b1_ext = nc.declare_dram_parameter("b1t", [P, N_PAIR], f32, isOutput=False)
    b2_ext = nc.declare_dram_parameter("b2t", [64, 4], f32, isOutput=False)
    b3_ext = nc.declare_dram_parameter("b3t", [P, N_OUT], f32, isOutput=False)
    out_ext = nc.declare_dram_parameter("out", [B_SHARD, N_OUT], f32,
                                        isOutput=True)

    with tile.TileContext(nc) as tc:
        with (
            tc.tile_pool(name="wpool", bufs=1) as wpool,
            tc.tile_pool(name="hp", bufs=12) as hp_pool,
            tc.tile_pool(name="gt", bufs=3) as gt_pool,
            tc.tile_pool(name="osb", bufs=3) as out_pool,
            tc.tile_pool(name="ps1", bufs=3, space="PSUM") as ps1_pool,
            tc.tile_pool(name="ps2", bufs=2, space="PSUM") as ps2_pool,
            tc.tile_pool(name="ps3", bufs=2, space="PSUM") as ps3_pool,
        ):
            w1sb = wpool.tile([P, N_BLK, P], wdt)
            nc.scalar.dma_start(out=w1sb[:], in_=w1_ext.rearrange("p (b j) -> p b j", j=P))
            w2sb = wpool.tile([P, 8, 64], f16)
            nc.scalar.dma_start(
                out=w2sb[:], in_=w2_ext.rearrange("p (b j) -> p b j", j=64))
            w3sb = wpool.tile([P, 2, N_OUT], f16)
            nc.scalar.dma_start(out=w3sb[:], in_=w3_ext.rearrange("p (b j) -> p b j", j=N_OUT))
            b1sb = wpool.tile([P, N_PAIR], f32)
            nc.scalar.dma_start(out=b1sb[:], in_=b1_ext[:])
            b2sb = wpool.tile([64, 4], f32)
            nc.scalar.dma_start(out=b2sb[:], in_=b2_ext[:])
            b3sb = wpool.tile([P, N_OUT], f32)
            nc.scalar.dma_start(out=b3sb[:], in_=b3_ext[:])

            # manual double-buffered x piece tiles: [set][stream][piece]
            # where set = unrolled-body parity. Writes into set s wait
            # (WAR) for the previous body-with-parity-s's consumers — so
            # the next body's x DMA overlaps this body's compute.
            sdefs = [(key, cols, dt)
                     for key, cols, dt in (("x16", s16, f16), ("x8", s8, f8))
                     if cols]
            xsets = []
            for s in range(2):
                streams = {}
                for key, cols, dt in sdefs:
                    streams[key] = [
                        wpool.tile([P, t1 - t0, N_CH, NB], dt,
                                   name=f"xp{s}_{key}_{pi}")
                        for pi, (t0, t1) in enumerate(_stream_pieces(cols))
                    ]
                xsets.append(streams)

            xt_fix = None
            if mode == "compute":
                # persistent x chunk for compute-only probe (chunk 0 data,
                # same per-col rhs dtypes as the real kernel)
                xt_fix = {}
                for key, cols, dt in sdefs:
                    xt_fix[key] = wpool.tile(
                        [P, len(cols), N_CH, NB], dt, name=f"xf_{key}")
                    nc.sync.dma_start(
                        out=xt_fix[key][:],
                        in_=x_exts[key][:, :len(cols) * N_CH * NB].rearrange(
                            "p (t c j) -> p t c j", c=N_CH, j=NB),
                    )
            if mode == "dma":
                # out is never written in the loop; write something once
                nc.scalar.dma_start(out=out_ext[0:P, :], in_=b3sb[:])

            import contextlib
            loop_cm = tc.For_i(0, n_iters, 1) if n_iters > 1 else contextlib.nullcontext()
            with loop_cm:
                for u in range(unroll):
                    _kernel_body(nc, tc, locals(), has_bias, mode,
                                 xt_fix, xsets[u % 2])

    nc.compile()
    return nc


def _kernel_body(nc, tc, env, has_bias, mode="full", xt_fix=None,
                 xset=None):
    x_exts = env["x_exts"]
    out_ext = env["out_ext"]
    s16, s8 = env["s16"], env["s8"]
    w1sb, w2sb, w3sb = env["w1sb"], env["w2sb"], env["w3sb"]
    b1sb, b2sb, b3sb = env["b1sb"], env["b2sb"], env["b3sb"]
    hp_pool, gt_pool, out_pool = env["hp_pool"], env["gt_pool"], env["out_pool"]
    ps1_pool, ps2_pool, ps3_pool = env["ps1_pool"], env["ps2_pool"], env["ps3_pool"]
    f16 = mybir.dt.float16
    f32 = mybir.dt.float32
    xqs = [getattr(nc, q) for q in XQ]
    # col -> (stream key, stream-col index)
    colmap = {}
    for key, cols in (("x16", s16), ("x8", s8)):
        for si, t in enumerate(cols):
            colmap[t] = (key, si)
    scols = {"x16": s16, "x8": s8}

    b0 = 0
    qi = 0
    for ch, nb in enumerate(CHUNKS):
        if mode == "compute":
            pieces = None
        else:
            # chunk ch of the pre-transposed stream-col-major x streams:
            # each piece is one fully contiguous DMA per partition
            pieces = {}
            for key, tiles in xset.items():
                cols = scols[key]
                c0 = len(cols) * N_CH * b0
                src = x_exts[key][:, c0:c0 + len(cols) * N_CH * nb].rearrange(
                    "p (t c j) -> p t c j", c=N_CH, j=nb
                )
                plist = []
                for pi, (t0, t1) in enumerate(_stream_pieces(cols)):
                    xp = tiles[pi][:, :t1 - t0, :, :nb]
                    xqs[qi % len(xqs)].dma_start(
                        out=xp[:], in_=src[:, t0:t1, :, :])
                    qi += 1
                    plist.append(xp)
                pieces[key] = plist
        if mode == "dma":
            b0 += nb
            continue

        def xt_rhs(c, t):
            key, si = colmap[t]
            if pieces is None:
                return xt_fix[key][:, si, c, :nb]
            pi, tl = si // PIECE_COLS, si % PIECE_COLS
            return pieces[key][pi][:, tl, c, :]

        hps = {}
        gt_t = gt_pool.tile([P, 2, NB], f16, name="gtt")
        gt = gt_t[:, :, :nb]
        for m in range(N_PAIR):
            # ---- layer 1, pair m: banded matmuls, t-major so the pair
            # starts as soon as its first tile-column lands ----
            ps_t = ps1_pool.tile([P, NB], f32, name="ps1t")
            ps = ps_t[:, :nb]
            mm_list = [(c, t) for t in _pair_tiles(m) for c in range(N_CH)]
            for i, (c, t) in enumerate(mm_list):
                nc.tensor.matmul(
                    ps[:],
                    w1sb[:, BLK_IDX[(m, c, t)], :],
                    xt_rhs(c, t),
                    start=(i == 0),
                    stop=(i == len(mm_list) - 1),
                )
            hp_t = hp_pool.tile([P, NB], f16, name="hpt")
            hp = hp_t[:, :nb]
            # alternate DVE / Act for the PSUM->SBUF h copies (GPSIMD/Pool
            # cannot read PSUM)
            if has_bias:
                if m % 2 == 0:
                    nc.vector.tensor_scalar_add(hp[:], ps[:], b1sb[:, m:m + 1])
                else:
                    nc.scalar.add(hp[:], ps[:], b1sb[:, m:m + 1])
            else:
                if m % 2 == 0:
                    nc.vector.tensor_copy(out=hp[:], in_=ps[:])
                else:
                    nc.scalar.copy(out=hp[:], in_=ps[:])
            hps[m] = hp

            # ---- layer 2, group g right after its two pairs ----
            if m % 2 == 1:
                g = m // 2
                ps2_t = ps2_pool.tile([64, NB], f32, name="ps2t")
                ps2 = ps2_t[:, :nb]
                for piece in range(2):
                    nc.tensor.matmul(
                        ps2[:],
                        w2sb[:, 2 * g + piece, :],
                        hps[2 * g + piece][:],
                        start=(piece == 0),
                        stop=(piece == 1),
                    )
                lo = 64 * (g % 2)
                if has_bias:
                    nc.vector.tensor_scalar_add(
                        gt[lo:lo + 64, g // 2], ps2[:], b2sb[:, g:g + 1],
                    )
                else:
                    nc.vector.tensor_copy(out=gt[lo:lo + 64, g // 2], in_=ps2[:])

        # ---- layer 3: back to batch-major ----
        nj = nb // P
        osb_t = out_pool.tile([P, NB // P, N_OUT], f32, name="osbt")
        osb = osb_t[:, :nj]
        for js in range(nj):
            ps3 = ps3_pool.tile([P, N_OUT], f32)
            for piece in range(2):
                nc.tensor.matmul(
                    ps3[:],
                    gt[:, piece, js * P:(js + 1) * P],
                    w3sb[:, piece, :],
                    start=(piece == 0),
                    stop=(piece == 1),
                )
            # spread the 4 osb copies over Act/DVE
            if has_bias:
                if js % 2 == 0:
                    nc.scalar.add(osb[:, js], ps3[:], b3sb[:])
                else:
                    nc.vector.tensor_tensor(
                        osb[:, js], ps3[:], b3sb[:], mybir.AluOpType.add,
                    )
            else:
                if js % 2 == 0:
                    nc.scalar.copy(out=osb[:, js], in_=ps3[:])
                else:
                    nc.vector.tensor_copy(out=osb[:, js], in_=ps3[:])
        nc.scalar.dma_start(
            out=out_ext[b0:b0 + nb, :].rearrange("(j p) n -> p j n", p=P),
            in_=osb[:],
        )
        b0 += nb


_CACHED_NC = None

# Store W1 as fp8 e3m4 (scaled by W1_SCALE into e3m4's dense range): cuts
# LDWEIGHTS stream time 2x via 4-byte FWL. 1/W1_SCALE folds into W2.
W1_F8 = True
W1_SCALE = 64.0


def _prep_in_maps(x, W1, b1, W2, b2, W3, b3):
    xf = np.asarray(x, dtype=np.float32)
    w1sb, w2sb, w3sb, b1t, b2t, b3t = _pack_weights(W1, b1, W2, b2, W3, b3)
    in_maps = []
    for i in range(N_CORES):
        m = {
            "w1": w1sb,
            "w2": w2sb,
            "w3": w3sb,
            "b1t": b1t,
            "b2t": b2t,
            "b3t": b3t,
        }
        m.update(_pack_x_streams(xf[i * B_SHARD:(i + 1) * B_SHARD]))
        in_maps.append(m)
    return in_maps


_CACHED_BIAS_NC = None


def kernel(x, W1, b1, W2, b2, W3, b3):
    global _CACHED_NC, _CACHED_BIAS_NC
    has_bias = bool(
        np.any(np.asarray(b1)) or np.any(np.asarray(b2)) or np.any(np.asarray(b3))
    )
    if has_bias:
        if _CACHED_BIAS_NC is None:
            _CACHED_BIAS_NC = build_kernel(has_bias=True)
        nc = _CACHED_BIAS_NC
    else:
        if _CACHED_NC is None:
            _CACHED_NC = build_kernel()
        nc = _CACHED_NC
    in_maps = _prep_in_maps(x, W1, b1, W2, b2, W3, b3)
    last_err = None
    for attempt in range(3):
        try:
            res = run_bass_kernel_spmd(nc, in_maps, core_ids=list(range(N_CORES)))
            break
        except Exception as e:  # transient device/axon failures
            last_err = e
            if attempt == 2:
                raise
            import time as _time
            _time.sleep(20.0)
    return np.concatenate([res.results[i]["out"] for i in range(N_CORES)], axis=0)


# revision 11
# speedup vs baseline: 1.2187x; 1.0383x over previous
"""Trainium2 Bass kernel for the windowed 3-channel MLP (dense_mlp).

Reference computation (B=8192):
  x [B, 6144] -> view [B, 3, 2048]
  16 overlapping windows/channel (len 256, stride 119)
  h[b,c,w,:] = win @ W1[c,w] + b1[c,w]          # [B,3,16,64]
  h = mean over c                               # [B,16,64]
  g[b,grp]   = h-grp(4 windows=256) @ W2[grp] + b2   # [B,4,64]
  out        = g.reshape(B,256) @ W3 + b3       # [B,255]

Strategy: pure data parallelism over 8 cores (B/8 = 1024 rows each).
x is cast fp16 and host-packed t-major feature-major so each k-piece is
one fully contiguous HBM->SBUF DMA. W1 is fp8 e3m4 (scaled x64, 1/64
folded into W2) to halve the LDWEIGHTS stream. f32 PSUM accumulate;
fp16 between layers.

On-device per core, per batch chunk of 512 (PSUM free-dim limit):
  - x streamed as N_PIECE tile-column pieces into per-chunk
    double-buffered piece tiles (manual 2-set rotation across unrolled
    loop bodies -> next body's DMA overlaps this body's compute).
  - Layer 1 banded matmuls per window pair, k-tiles consumed t-major so
    pair m starts as soon as its first tile-column lands.
  - L2 group g emitted right after pair 2g+1 (fills DMA-lag stalls and
    shrinks the tail); L3 + per-chunk contiguous out DMA at chunk end.
  - PSUM->SBUF copies spread over DVE/Pool/Act so no engine serializes
    the tail.
Bench loop: UNROLL bodies per tc.For_i iteration to amortize the
all-engine barrier at the loop back-edge.
"""

import sys

sys.path.insert(0, "/opt/trn_rl_repo")

import numpy as np

import concourse.bass as bass
import concourse.mybir as mybir
import concourse.tile as tile
from concourse import bacc
from concourse.bass_utils import run_bass_kernel_spmd

P = 128
N_CORES = 8
B_FULL = 8192
B_SHARD = B_FULL // N_CORES          # 1024
CH_LEN = 2048
N_CH = 3
K_FULL = N_CH * CH_LEN               # 6144
N_WIN = 16
WIN = 256
STRIDE = 119
N_PAIR = 8                           # window pairs (2 windows x 64 = 128 feats)
KT_CH = CH_LEN // P                  # 16 k-tiles (tile-columns) per channel
KT_ALL = K_FULL // P                 # 48
NB = 512                             # max batch chunk (matmul free dim)
CHUNKS = [512, 512]                  # batch chunk sizes (sum = B_SHARD)
assert sum(CHUNKS) == B_SHARD
assert all(nb % P == 0 for nb in CHUNKS)
N_OUT = 255

# Pair-aligned banding: ship each pair's 375-row band as 3 k-tiles aligned
# at row 238m (x rows duplicated across pairs, +50% x bytes — affordable
# at fp8), cutting layer 1 from 90 grid-aligned blocks to 72 and the PE
# cycle floor by ~17%.
ALIGN_PAIRS = True
N_ATILE = 3                          # aligned k-tiles per pair band

# ---- tunables ----
# tile-columns (0..15 per channel) whose x is shipped fp8 e3m4 instead of
# fp16. DMA-bound kernel: each fp8 col saves 1/32 of x traffic at the cost
# of quantization error (~1.9e-2 end-to-end rel err at all-16 fp8,
# ~1.66e-2 at the default odd-col half split, 1.35e-2 at none).
X8_COLS = tuple(range(24))
KT_EFF = N_PAIR * N_ATILE if ALIGN_PAIRS else KT_CH  # x tile-cols/chunk
# piece size (stream tile-cols per DMA) for the x streams; 3 in aligned
# mode so piece m carries exactly pair m's band
PIECE_COLS = 3 if ALIGN_PAIRS else 2
# queues to issue x piece DMAs from, round-robin. NOTE: keep x DMAs off
# the scalar/Act queue: Act's engine exec-queue depth is 0, so its
# sequencer blocks on each PSUM copy and any DMA issued behind those
# copies loses its prefetch lead (measured: no speedup from 25% fewer
# DMA bytes until x DMAs moved off Act).
XQ = ["sync"]
# unrolled bodies per For_i iteration in the bench loop
UNROLL = 8


def _streams():
    """(cols_16, cols_8) tile-col lists per stream."""
    s8 = sorted(c for c in X8_COLS if c < KT_EFF)
    s16 = [t for t in range(KT_EFF) if t not in s8]
    return s16, s8


def _col_rows(col):
    """x row range [r0, r1) within a channel for tile-col `col`."""
    if ALIGN_PAIRS:
        m, tau = divmod(col, N_ATILE)
        r0 = 2 * STRIDE * m + P * tau
    else:
        r0 = P * col
    return r0, r0 + P


def _stream_pieces(cols):
    n = len(cols)
    return [(i, min(i + PIECE_COLS, n)) for i in range(0, n, PIECE_COLS)]


def _pair_tiles(m):
    """k-tiles of one channel that intersect window pair m (rows 238m..238m+374)."""
    lo = (2 * STRIDE * m) // P
    hi = (2 * STRIDE * m + 2 * STRIDE + WIN - 2 - STRIDE) // P  # (238m+374)//128
    return list(range(lo, min(hi, KT_CH - 1) + 1))


if ALIGN_PAIRS:
    # Block order for layer-1 packed weights: (m, c, tau)
    BLOCKS = [(m, c, tau) for m in range(N_PAIR) for c in range(N_CH)
              for tau in range(N_ATILE)]
else:
    # for m, for c, for t (grid-aligned k-tiles)
    BLOCKS = [(m, c, t) for m in range(N_PAIR) for c in range(N_CH)
              for t in _pair_tiles(m)]
BLK_IDX = {key: i for i, key in enumerate(BLOCKS)}
N_BLK = len(BLOCKS)                  # 72 aligned / 90 grid


def _pack_weights(W1, b1, W2, b2, W3, b3):
    """Host-side packing of the tiny weight tensors into device layouts.

    W1 is scaled by W1_SCALE and stored e3m4 (layer-1 outputs come out
    scaled; 1/W1_SCALE is folded into W2, and W1_SCALE into b1).
    """
    W1 = np.asarray(W1, dtype=np.float32)
    ki = np.arange(P)[:, None]                    # tile-local k row
    j = np.arange(P)[None, :]                     # pair-local output feature
    w_off = j // 64                               # window within pair
    n = j % 64

    w1p = np.zeros((N_BLK, P, P), dtype=np.float32)
    for i, (m, c, t) in enumerate(BLOCKS):
        w = 2 * m + w_off                         # [1,128] window index
        base = (2 * STRIDE * m + P * t) if ALIGN_PAIRS else (P * t)
        koff = base + ki - STRIDE * w             # [128,128] k within window
        mask = (koff >= 0) & (koff < WIN)
        w1p[i] = np.where(
            mask, W1[c, w, np.clip(koff, 0, WIN - 1), n] / 3.0, 0.0
        )
    # device layout: [P(ki), N_BLK * P(j)] contiguous per partition
    w1flat = np.ascontiguousarray(w1p.transpose(1, 0, 2).reshape(P, N_BLK * P))
    if W1_F8:
        import ml_dtypes
        w1sb = (w1flat * W1_SCALE).astype(ml_dtypes.float8_e3m4)
    else:
        w1sb = w1flat.astype(np.float16)

    # W2 [4,256,64] -> pieces [g,p][128,64] -> [P, 8, 64]
    w2p = np.asarray(W2, dtype=np.float32).reshape(4, 2, P, 64)
    if W1_F8:
        w2p = w2p / W1_SCALE
    w2sb = np.ascontiguousarray(
        w2p.transpose(2, 0, 1, 3).reshape(P, 8 * 64)
    ).astype(np.float16)

    # W3 [256,255] -> [P, 2, 255]
    w3p = np.asarray(W3, dtype=np.float32).reshape(2, P, N_OUT)
    w3sb = np.ascontiguousarray(
        w3p.transpose(1, 0, 2).reshape(P, 2 * N_OUT)
    ).astype(np.float16)

    # biases (per-partition layouts)
    b1m = np.asarray(b1, dtype=np.float32).mean(axis=0)        # [16,64]
    if W1_F8:
        b1m = b1m * W1_SCALE
    b1t = np.ascontiguousarray(b1m.reshape(N_PAIR, P).T)       # [128, 8]
    b2t = np.ascontiguousarray(np.asarray(b2, dtype=np.float32).T)  # [64, 4]
    b3t = np.ascontiguousarray(
        np.broadcast_to(np.asarray(b3, dtype=np.float32), (P, N_OUT))
    )                                                          # [128, 255]
    return w1sb, w2sb, w3sb, b1t, b2t, b3t


def _pack_x_streams(x_shard):
    """[1024, 6144] f32 -> two chunk-major tile-col-major streams.

    Per chunk the block [128, len(cols)*3*nb] holds xT[k, b] laid out
    stream-col-major then channel then batch, so a k-piece (stream-col
    range) is one fully contiguous run per partition. Stream 16 carries
    cols not in X8_COLS as fp16; stream 8 carries X8_COLS as fp8 e3m4.
    """
    import ml_dtypes
    s16, s8 = _streams()
    out = {}
    for key, cols, dt in (("x16", s16, np.float16),
                          ("x8", s8, ml_dtypes.float8_e3m4)):
        if not cols:
            continue
        parts = []
        b0 = 0
        for nb in CHUNKS:
            xc = x_shard[b0:b0 + nb].reshape(nb, N_CH, CH_LEN)
            blk = np.zeros((P, len(cols), N_CH, nb), np.float32)
            for i, col in enumerate(cols):
                r0, r1 = _col_rows(col)
                r1c = min(r1, CH_LEN)
                blk[:r1c - r0, i] = xc[:, :, r0:r1c].transpose(2, 1, 0)
            parts.append(blk.reshape(P, len(cols) * N_CH * nb))
            b0 += nb
        out[key] = np.ascontiguousarray(
            np.concatenate(parts, axis=1)).astype(dt)
    return out


def build_kernel(reps=1, has_bias=False, mode="full", unroll=None):
    if unroll is None:
        unroll = 1 if reps == 1 else UNROLL
    assert reps % unroll == 0
    n_iters = reps // unroll
    nc = bacc.Bacc("TRN2", target_bir_lowering=False, debug=False,
                   num_devices=N_CORES)
    f16 = mybir.dt.float16
    f32 = mybir.dt.float32
    f8 = mybir.dt.float8e3
    s16, s8 = _streams()

    wdt = mybir.dt.float8e3 if W1_F8 else f16
    x_exts = {}
    if s16:
        x_exts["x16"] = nc.declare_dram_parameter(
            "x16", [P, len(s16) * N_CH * B_SHARD], f16, isOutput=False)
    if s8:
        x_exts["x8"] = nc.declare_dram_parameter(
            "x8", [P, len(s8) * N_CH * B_SHARD], f8, isOutput=False)
    w1_ext = nc.declare_dram_parameter("w1", [P, N_BLK * P], wdt, isOutput=False)
    w2_ext = nc.declare_dram_parameter("w2", [P, 8 * 64], f16, isOutput=False)
    w3_ext = nc.declare_dram_parameter("w3", [P, 2 * N_OUT], f16, isOutput=False)
    b1_ext = nc.declare_dram_parameter("b1t", [P, N_PAIR], f32, isOutput=False)
    b2_ext = nc.declare_dram_parameter("b2t", [64, 4], f32, isOutput=False)
    b3_ext = nc.declare_dram_parameter("b3t", [P, N_OUT], f32, isOutput=False)
    out_ext = nc.declare_dram_parameter("out", [B_SHARD, N_OUT], f32,
                                        isOutput=True)

    with tile.TileContext(nc) as tc:
        with (
            tc.tile_pool(name="wpool", bufs=1) as wpool,
            tc.tile_pool(name="hp", bufs=12) as hp_pool,
            tc.tile_pool(name="gt", bufs=3) as gt_pool,
            tc.tile_pool(name="osb", bufs=3) as out_pool,
            tc.tile_pool(name="ps1", bufs=3, space="PSUM") as ps1_pool,
            tc.tile_pool(name="ps2", bufs=2, space="PSUM") as ps2_pool,
            tc.tile_pool(name="ps3", bufs=2, space="PSUM") as ps3_pool,
        ):
            w1sb = wpool.tile([P, N_BLK, P], wdt)
            nc.scalar.dma_start(out=w1sb[:], in_=w1_ext.rearrange("p (b j) -> p b j", j=P))
            w2sb = wpool.tile([P, 8, 64], f16)
            nc.scalar.dma_start(
                out=w2sb[:], in_=w2_ext.rearrange("p (b j) -> p b j", j=64))
            w3sb = wpool.tile([P, 2, N_OUT], f16)
            nc.scalar.dma_start(out=w3sb[:], in_=w3_ext.rearrange("p (b j) -> p b j", j=N_OUT))
            b1sb = wpool.tile([P, N_PAIR], f32)
            nc.scalar.dma_start(out=b1sb[:], in_=b1_ext[:])
            b2sb = wpool.tile([64, 4], f32)
            nc.scalar.dma_start(out=b2sb[:], in_=b2_ext[:])
            b3sb = wpool.tile([P, N_OUT], f32)
            nc.scalar.dma_start(out=b3sb[:], in_=b3_ext[:])

            # manual double-buffered x piece tiles: [set][stream][piece]
            # where set = unrolled-body parity. Writes into set s wait
            # (WAR) for the previous body-with-parity-s's consumers — so
            # the next body's x DMA overlaps this body's compute.
            sdefs = [(key, cols, dt)
                     for key, cols, dt in (("x16", s16, f16), ("x8", s8, f8))
                     if cols]
            xsets = []
            for s in range(2):
                streams = {}
                for key, cols, dt in sdefs:
                    streams[key] = [
                        wpool.tile([P, t1 - t0, N_CH, NB], dt,
                                   name=f"xp{s}_{key}_{pi}")
                        for pi, (t0, t1) in enumerate(_stream_pieces(cols))
                    ]
                xsets.append(streams)

            xt_fix = None
            if mode == "compute":
                # persistent x chunk for compute-only probe (chunk 0 data,
                # same per-col rhs dtypes as the real kernel)
                xt_fix = {}
                for key, cols, dt in sdefs:
                    xt_fix[key] = wpool.tile(
                        [P, len(cols), N_CH, NB], dt, name=f"xf_{key}")
                    nc.sync.dma_start(
                        out=xt_fix[key][:],
                        in_=x_exts[key][:, :len(cols) * N_CH * NB].rearrange(
                            "p (t c j) -> p t c j", c=N_CH, j=NB),
                    )
            if mode == "dma":
                # out is never written in the loop; write something once
                nc.scalar.dma_start(out=out_ext[0:P, :], in_=b3sb[:])

            import contextlib
            loop_cm = tc.For_i(0, n_iters, 1) if n_iters > 1 else contextlib.nullcontext()
            with loop_cm:
                for u in range(unroll):
                    _kernel_body(nc, tc, locals(), has_bias, mode,
                                 xt_fix, xsets[u % 2])

    nc.compile()
    return nc


def _kernel_body(nc, tc, env, has_bias, mode="full", xt_fix=None,
                 xset=None):
    x_exts = env["x_exts"]
    out_ext = env["out_ext"]
    s16, s8 = env["s16"], env["s8"]
    w1sb, w2sb, w3sb = env["w1sb"], env["w2sb"], env["w3sb"]
    b1sb, b2sb, b3sb = env["b1sb"], env["b2sb"], env["b3sb"]
    hp_pool, gt_pool, out_pool = env["hp_pool"], env["gt_pool"], env["out_pool"]
    ps1_pool, ps2_pool, ps3_pool = env["ps1_pool"], env["ps2_pool"], env["ps3_pool"]
    f16 = mybir.dt.float16
    f32 = mybir.dt.float32
    xqs = [getattr(nc, q) for q in XQ]
    # col -> (stream key, stream-col index)
    colmap = {}
    for key, cols in (("x16", s16), ("x8", s8)):
        for si, t in enumerate(cols):
            colmap[t] = (key, si)
    scols = {"x16": s16, "x8": s8}

    b0 = 0
    qi = 0
    for ch, nb in enumerate(CHUNKS):
        if mode == "compute":
            pieces = None
        else:
            # chunk ch of the pre-transposed stream-col-major x streams:
            # each piece is one fully contiguous DMA per partition
            pieces = {}
            for key, tiles in xset.items():
                cols = scols[key]
                c0 = len(cols) * N_CH * b0
                src = x_exts[key][:, c0:c0 + len(cols) * N_CH * nb].rearrange(
                    "p (t c j) -> p t c j", c=N_CH, j=nb
                )
                plist = []
                for pi, (t0, t1) in enumerate(_stream_pieces(cols)):
                    xp = tiles[pi][:, :t1 - t0, :, :nb]
                    xqs[qi % len(xqs)].dma_start(
                        out=xp[:], in_=src[:, t0:t1, :, :])
                    qi += 1
                    plist.append(xp)
                pieces[key] = plist
        if mode == "dma":
            b0 += nb
            continue

        def xt_rhs(c, t):
            key, si = colmap[t]
            if pieces is None:
                return xt_fix[key][:, si, c, :nb]
            pi, tl = si // PIECE_COLS, si % PIECE_COLS
            return pieces[key][pi][:, tl, c, :]

        hps = {}
        gt_t = gt_pool.tile([P, 2, NB], f16, name="gtt")
        gt = gt_t[:, :, :nb]
        for m in range(N_PAIR):
            # ---- layer 1, pair m: banded matmuls, t-major so the pair
            # starts as soon as its first tile-column lands ----
            ps_t = ps1_pool.tile([P, NB], f32, name="ps1t")
            ps = ps_t[:, :nb]
            if ALIGN_PAIRS:
                mm_list = [(c, tau) for tau in range(N_ATILE)
                           for c in range(N_CH)]
                cols_of = lambda tau: N_ATILE * m + tau
            else:
                mm_list = [(c, t) for t in _pair_tiles(m) for c in range(N_CH)]
                cols_of = lambda t: t
            for i, (c, t) in enumerate(mm_list):
                nc.tensor.matmul(
                    ps[:],
                    w1sb[:, BLK_IDX[(m, c, t)], :],
                    xt_rhs(c, cols_of(t)),
                    start=(i == 0),
                    stop=(i == len(mm_list) - 1),
                )
            hp_t = hp_pool.tile([P, NB], f16, name="hpt")
            hp = hp_t[:, :nb]
            # alternate DVE / Act for the PSUM->SBUF h copies (GPSIMD/Pool
            # cannot read PSUM)
            if has_bias:
                if m % 2 == 0:
                    nc.vector.tensor_scalar_add(hp[:], ps[:], b1sb[:, m:m + 1])
                else:
                    nc.scalar.add(hp[:], ps[:], b1sb[:, m:m + 1])
            else:
                if m % 2 == 0:
                    nc.vector.tensor_copy(out=hp[:], in_=ps[:])
                else:
                    nc.scalar.copy(out=hp[:], in_=ps[:])
            hps[m] = hp

            # ---- layer 2, group g right after its two pairs ----
            if m % 2 == 1:
                g = m // 2
                ps2_t = ps2_pool.tile([64, NB], f32, name="ps2t")
                ps2 = ps2_t[:, :nb]
                for piece in range(2):
                    nc.tensor.matmul(
                        ps2[:],
                        w2sb[:, 2 * g + piece, :],
                        hps[2 * g + piece][:],
                        start=(piece == 0),
                        stop=(piece == 1),
                    )
                lo = 64 * (g % 2)
                if has_bias:
                    nc.vector.tensor_scalar_add(
                        gt[lo:lo + 64, g // 2], ps2[:], b2sb[:, g:g + 1],
                    )
                else:
                    nc.vector.tensor_copy(out=gt[lo:lo + 64, g // 2], in_=ps2[:])

        # ---- layer 3: back to batch-major ----
        nj = nb // P
        osb_t = out_pool.tile([P, NB // P, N_OUT], f32, name="osbt")
        osb = osb_t[:, :nj]
        for js in range(nj):
            ps3 = ps3_pool.tile([P, N_OUT], f32)
            for piece in range(2):
                nc.tensor.matmul(
                    ps3[:],
                    gt[:, piece, js * P:(js + 1) * P],
                    w3sb[:, piece, :],
                    start=(piece == 0),
                    stop=(piece == 1),
                )
            # spread the 4 osb copies over Act/DVE
            if has_bias:
                if js % 2 == 0:
                    nc.scalar.add(osb[:, js], ps3[:], b3sb[:])
                else:
                    nc.vector.tensor_tensor(
                        osb[:, js], ps3[:], b3sb[:], mybir.AluOpType.add,
                    )
            else:
                if js % 2 == 0:
                    nc.scalar.copy(out=osb[:, js], in_=ps3[:])
                else:
                    nc.vector.tensor_copy(out=osb[:, js], in_=ps3[:])
        nc.scalar.dma_start(
            out=out_ext[b0:b0 + nb, :].rearrange("(j p) n -> p j n", p=P),
            in_=osb[:],
        )
        b0 += nb


_CACHED_NC = None

# Store W1 as fp8 e3m4 (scaled by W1_SCALE into e3m4's dense range).
# With x shipped fp8 (X8_COLS=all), W1 stays fp16: the error budget goes
# to x, and LDWEIGHTS showed no measurable serialization on this HW
# (compute-only probe ran at the matmul cycle floor).
W1_F8 = False
W1_SCALE = 64.0


def _prep_in_maps(x, W1, b1, W2, b2, W3, b3):
    xf = np.asarray(x, dtype=np.float32)
    w1sb, w2sb, w3sb, b1t, b2t, b3t = _pack_weights(W1, b1, W2, b2, W3, b3)
    in_maps = []
    for i in range(N_CORES):
        m = {
            "w1": w1sb,
            "w2": w2sb,
            "w3": w3sb,
            "b1t": b1t,
            "b2t": b2t,
            "b3t": b3t,
        }
        m.update(_pack_x_streams(xf[i * B_SHARD:(i + 1) * B_SHARD]))
        in_maps.append(m)
    return in_maps


_CACHED_BIAS_NC = None


def kernel(x, W1, b1, W2, b2, W3, b3):
    global _CACHED_NC, _CACHED_BIAS_NC
    has_bias = bool(
        np.any(np.asarray(b1)) or np.any(np.asarray(b2)) or np.any(np.asarray(b3))
    )
    if has_bias:
        if _CACHED_BIAS_NC is None:
            _CACHED_BIAS_NC = build_kernel(has_bias=True)
        nc = _CACHED_BIAS_NC
    else:
        if _CACHED_NC is None:
            _CACHED_NC = build_kernel()
        nc = _CACHED_NC
    in_maps = _prep_in_maps(x, W1, b1, W2, b2, W3, b3)
    last_err = None
    for attempt in range(3):
        try:
            res = run_bass_kernel_spmd(nc, in_maps, core_ids=list(range(N_CORES)))
            break
        except Exception as e:  # transient device/axon failures
            last_err = e
            if attempt == 2:
                raise
            import time as _time
            _time.sleep(20.0)
    return np.concatenate([res.results[i]["out"] for i in range(N_CORES)], axis=0)


# revision 14
# speedup vs baseline: 1.3744x; 1.1277x over previous
"""Trainium2 Bass kernel for the windowed 3-channel MLP (dense_mlp).

Reference computation (B=8192):
  x [B, 6144] -> view [B, 3, 2048]
  16 overlapping windows/channel (len 256, stride 119)
  h[b,c,w,:] = win @ W1[c,w] + b1[c,w]          # [B,3,16,64]
  h = mean over c                               # [B,16,64]
  g[b,grp]   = h-grp(4 windows=256) @ W2[grp] + b2   # [B,4,64]
  out        = g.reshape(B,256) @ W3 + b3       # [B,255]

Strategy: pure data parallelism over 8 cores (B/8 = 1024 rows each),
two batch chunks of 512 (PSUM free-dim limit) per core.

Key measured facts driving the design (all via looped-NEFF deltas on HW):
  - The PE matmul cycle floor dominates: cycles = #matmuls x free-dim.
    Pair-ALIGNED banding ships each window-pair's 375-row band as 3
    k-tiles aligned at row 238m (x rows duplicated across pairs, +50%
    x bytes) cutting layer 1 from 90 grid-aligned blocks to 72
    (measured -7.6us, exactly the cycle model's prediction).
  - fp16 LDWEIGHTS costs ~53ns serialized per matmul; fp8 e3m4 W1
    (scaled x64, 1/64 folded into W2) makes it ~free (~8.5us/body).
  - DMA sustains ~338 GB/s from one queue; x is shipped mostly fp8:
    14 of 24 aligned tile-cols fp8 e3m4, 10 fp16 (pair 7 + tau=1 cols;
    pair 0 all-fp8 so the post-barrier fill is small). End-to-end rel
    err 1.62e-2 vs the 2e-2 gate (x-quant + W1-quant budgets add in
    quadrature; host-emulated and HW-confirmed).
  - Keep x DMAs OFF the scalar/Act queue: Act's engine exec-queue depth
    is 0, so its sequencer blocks on each PSUM copy and DMAs issued
    behind those copies lose their prefetch lead. x pieces go on
    SP + Pool(gpsimd) queues; only the out DMA rides Act.
  - tc.For_i has an all-engine barrier per iteration (expensive on HW):
    the bench loop unrolls UNROLL bodies per iteration, with x piece
    tiles in a manual 2-set rotation so body u+1's DMAs (WAR on body
    u-1's consumers) prefetch during body u's compute.
  - PSUM->SBUF copies alternate DVE / Act engines; layer-2 group g is
    emitted right after pair 2g+1 to shrink the tail.
"""

import sys

sys.path.insert(0, "/opt/trn_rl_repo")

import numpy as np

import concourse.bass as bass
import concourse.mybir as mybir
import concourse.tile as tile
from concourse import bacc
from concourse.bass_utils import run_bass_kernel_spmd

P = 128
N_CORES = 8
B_FULL = 8192
B_SHARD = B_FULL // N_CORES          # 1024
CH_LEN = 2048
N_CH = 3
K_FULL = N_CH * CH_LEN               # 6144
N_WIN = 16
WIN = 256
STRIDE = 119
N_PAIR = 8                           # window pairs (2 windows x 64 = 128 feats)
KT_CH = CH_LEN // P                  # 16 k-tiles (tile-columns) per channel
KT_ALL = K_FULL // P                 # 48
NB = 512                             # max batch chunk (matmul free dim)
CHUNKS = [512, 512]                  # batch chunk sizes (sum = B_SHARD)
assert sum(CHUNKS) == B_SHARD
assert all(nb % P == 0 for nb in CHUNKS)
N_OUT = 255

# Pair-aligned banding: ship each pair's 375-row band as 3 k-tiles aligned
# at row 238m (x rows duplicated across pairs, +50% x bytes — affordable
# at fp8), cutting layer 1 from 90 grid-aligned blocks to 72 and the PE
# cycle floor by ~17%.
ALIGN_PAIRS = True
N_ATILE = 3                          # aligned k-tiles per pair band

# ---- tunables ----
# tile-columns (0..15 per channel) whose x is shipped fp8 e3m4 instead of
# fp16. DMA-bound kernel: each fp8 col saves 1/32 of x traffic at the cost
# of quantization error (~1.9e-2 end-to-end rel err at all-16 fp8,
# ~1.66e-2 at the default odd-col half split, 1.35e-2 at none).
# fp16 cols: tau=1 of pairs 0-6, plus all of pair 7 — pair 0 stays fully
# fp8 so the post-barrier fill isn't gated on a big fp16 piece.
X8_COLS = tuple(c for c in range(24) if c not in
                (1, 4, 7, 10, 13, 16, 19, 21, 22, 23))
KT_EFF = N_PAIR * N_ATILE if ALIGN_PAIRS else KT_CH  # x tile-cols/chunk
# piece size (stream tile-cols per DMA) for the x streams
PIECE_COLS = 3
# queues to issue x piece DMAs from, round-robin. NOTE: keep x DMAs off
# the scalar/Act queue: Act's engine exec-queue depth is 0, so its
# sequencer blocks on each PSUM copy and any DMA issued behind those
# copies loses its prefetch lead (measured: no speedup from 25% fewer
# DMA bytes until x DMAs moved off Act).
XQ = ["sync", "gpsimd"]
# unrolled bodies per For_i iteration in the bench loop
UNROLL = 8


def _streams():
    """(cols_16, cols_8) tile-col lists per stream."""
    s8 = sorted(c for c in X8_COLS if c < KT_EFF)
    s16 = [t for t in range(KT_EFF) if t not in s8]
    return s16, s8


def _col_rows(col):
    """x row range [r0, r1) within a channel for tile-col `col`."""
    if ALIGN_PAIRS:
        m, tau = divmod(col, N_ATILE)
        r0 = 2 * STRIDE * m + P * tau
    else:
        r0 = P * col
    return r0, r0 + P


def _stream_pieces(cols):
    n = len(cols)
    return [(i, min(i + PIECE_COLS, n)) for i in range(0, n, PIECE_COLS)]


def _pair_tiles(m):
    """k-tiles of one channel that intersect window pair m (rows 238m..238m+374)."""
    lo = (2 * STRIDE * m) // P
    hi = (2 * STRIDE * m + 2 * STRIDE + WIN - 2 - STRIDE) // P  # (238m+374)//128
    return list(range(lo, min(hi, KT_CH - 1) + 1))


if ALIGN_PAIRS:
    # Block order for layer-1 packed weights: (m, c, tau)
    BLOCKS = [(m, c, tau) for m in range(N_PAIR) for c in range(N_CH)
              for tau in range(N_ATILE)]
else:
    # for m, for c, for t (grid-aligned k-tiles)
    BLOCKS = [(m, c, t) for m in range(N_PAIR) for c in range(N_CH)
              for t in _pair_tiles(m)]
BLK_IDX = {key: i for i, key in enumerate(BLOCKS)}
N_BLK = len(BLOCKS)                  # 72 aligned / 90 grid


def _pack_weights(W1, b1, W2, b2, W3, b3):
    """Host-side packing of the tiny weight tensors into device layouts.

    W1 is scaled by W1_SCALE and stored e3m4 (layer-1 outputs come out
    scaled; 1/W1_SCALE is folded into W2, and W1_SCALE into b1).
    """
    W1 = np.asarray(W1, dtype=np.float32)
    ki = np.arange(P)[:, None]                    # tile-local k row
    j = np.arange(P)[None, :]                     # pair-local output feature
    w_off = j // 64                               # window within pair
    n = j % 64

    w1p = np.zeros((N_BLK, P, P), dtype=np.float32)
    for i, (m, c, t) in enumerate(BLOCKS):
        w = 2 * m + w_off                         # [1,128] window index
        base = (2 * STRIDE * m + P * t) if ALIGN_PAIRS else (P * t)
        koff = base + ki - STRIDE * w             # [128,128] k within window
        mask = (koff >= 0) & (koff < WIN)
        w1p[i] = np.where(
            mask, W1[c, w, np.clip(koff, 0, WIN - 1), n] / 3.0, 0.0
        )
    # device layout: [P(ki), N_BLK * P(j)] contiguous per partition
    w1flat = np.ascontiguousarray(w1p.transpose(1, 0, 2).reshape(P, N_BLK * P))
    if W1_F8:
        import ml_dtypes
        w1sb = (w1flat * W1_SCALE).astype(ml_dtypes.float8_e3m4)
    else:
        w1sb = w1flat.astype(np.float16)

    # W2 [4,256,64] -> pieces [g,p][128,64] -> [P, 8, 64]
    w2p = np.asarray(W2, dtype=np.float32).reshape(4, 2, P, 64)
    if W1_F8:
        w2p = w2p / W1_SCALE
    w2sb = np.ascontiguousarray(
        w2p.transpose(2, 0, 1, 3).reshape(P, 8 * 64)
    ).astype(np.float16)

    # W3 [256,255] -> [P, 2, 255]
    w3p = np.asarray(W3, dtype=np.float32).reshape(2, P, N_OUT)
    w3sb = np.ascontiguousarray(
        w3p.transpose(1, 0, 2).reshape(P, 2 * N_OUT)
    ).astype(np.float16)

    # biases (per-partition layouts)
    b1m = np.asarray(b1, dtype=np.float32).mean(axis=0)        # [16,64]
    if W1_F8:
        b1m = b1m * W1_SCALE
    b1t = np.ascontiguousarray(b1m.reshape(N_PAIR, P).T)       # [128, 8]
    b2t = np.ascontiguousarray(np.asarray(b2, dtype=np.float32).T)  # [64, 4]
    b3t = np.ascontiguousarray(
        np.broadcast_to(np.asarray(b3, dtype=np.float32), (P, N_OUT))
    )                                                          # [128, 255]
    return w1sb, w2sb, w3sb, b1t, b2t, b3t


def _pack_x_streams(x_shard):
    """[1024, 6144] f32 -> two chunk-major tile-col-major streams.

    Per chunk the block [128, len(cols)*3*nb] holds xT[k, b] laid out
    stream-col-major then channel then batch, so a k-piece (stream-col
    range) is one fully contiguous run per partition. Stream 16 carries
    cols not in X8_COLS as fp16; stream 8 carries X8_COLS as fp8 e3m4.
    """
    import ml_dtypes
    s16, s8 = _streams()
    out = {}
    for key, cols, dt in (("x16", s16, np.float16),
                          ("x8", s8, ml_dtypes.float8_e3m4)):
        if not cols:
            continue
        parts = []
        b0 = 0
        for nb in CHUNKS:
            xc = x_shard[b0:b0 + nb].reshape(nb, N_CH, CH_LEN)
            blk = np.zeros((P, len(cols), N_CH, nb), np.float32)
            for i, col in enumerate(cols):
                r0, r1 = _col_rows(col)
                r1c = min(r1, CH_LEN)
                blk[:r1c - r0, i] = xc[:, :, r0:r1c].transpose(2, 1, 0)
            parts.append(blk.reshape(P, len(cols) * N_CH * nb))
            b0 += nb
        out[key] = np.ascontiguousarray(
            np.concatenate(parts, axis=1)).astype(dt)
    return out


def build_kernel(reps=1, has_bias=False, mode="full", unroll=None):
    if unroll is None:
        unroll = 1 if reps == 1 else UNROLL
    assert reps % unroll == 0
    n_iters = reps // unroll
    nc = bacc.Bacc("TRN2", target_bir_lowering=False, debug=False,
                   num_devices=N_CORES)
    f16 = mybir.dt.float16
    f32 = mybir.dt.float32
    f8 = mybir.dt.float8e3
    s16, s8 = _streams()

    wdt = mybir.dt.float8e3 if W1_F8 else f16
    x_exts = {}
    if s16:
        x_exts["x16"] = nc.declare_dram_parameter(
            "x16", [P, len(s16) * N_CH * B_SHARD], f16, isOutput=False)
    if s8:
        x_exts["x8"] = nc.declare_dram_parameter(
            "x8", [P, len(s8) * N_CH * B_SHARD], f8, isOutput=False)
    w1_ext = nc.declare_dram_parameter("w1", [P, N_BLK * P], wdt, isOutput=False)
    w2_ext = nc.declare_dram_parameter("w2", [P, 8 * 64], f16, isOutput=False)
    w3_ext = nc.declare_dram_parameter("w3", [P, 2 * N_OUT], f16, isOutput=False)
    b1_ext = nc.declare_dram_parameter("b1t", [P, N_PAIR], f32, isOutput=False)
    b2_ext = nc.declare_dram_parameter("b2t", [64, 4], f32, isOutput=False)
    b3_ext = nc.declare_dram_parameter("b3t", [P, N_OUT], f32, isOutput=False)
    out_ext = nc.declare_dram_parameter("out", [B_SHARD, N_OUT], f32,
                                        isOutput=True)

    with tile.TileContext(nc) as tc:
        with (
            tc.tile_pool(name="wpool", bufs=1) as wpool,
            tc.tile_pool(name="hp", bufs=12) as hp_pool,
            tc.tile_pool(name="gt", bufs=3) as gt_pool,
            tc.tile_pool(name="osb", bufs=3) as out_pool,
            tc.tile_pool(name="ps1", bufs=3, space="PSUM") as ps1_pool,
            tc.tile_pool(name="ps2", bufs=2, space="PSUM") as ps2_pool,
            tc.tile_pool(name="ps3", bufs=2, space="PSUM") as ps3_pool,
        ):
            w1sb = wpool.tile([P, N_BLK, P], wdt)
            nc.scalar.dma_start(out=w1sb[:], in_=w1_ext.rearrange("p (b j) -> p b j", j=P))
            w2sb = wpool.tile([P, 8, 64], f16)
            nc.scalar.dma_start(
                out=w2sb[:], in_=w2_ext.rearrange("p (b j) -> p b j", j=64))
            w3sb = wpool.tile([P, 2, N_OUT], f16)
            nc.scalar.dma_start(out=w3sb[:], in_=w3_ext.rearrange("p (b j) -> p b j", j=N_OUT))
            b1sb = wpool.tile([P, N_PAIR], f32)
            nc.scalar.dma_start(out=b1sb[:], in_=b1_ext[:])
            b2sb = wpool.tile([64, 4], f32)
            nc.scalar.dma_start(out=b2sb[:], in_=b2_ext[:])
            b3sb = wpool.tile([P, N_OUT], f32)
            nc.scalar.dma_start(out=b3sb[:], in_=b3_ext[:])

            # manual double-buffered x piece tiles: [set][stream][piece]
            # where set = unrolled-body parity. Writes into set s wait
            # (WAR) for the previous body-with-parity-s's consumers — so
            # the next body's x DMA overlaps this body's compute.
            sdefs = [(key, cols, dt)
                     for key, cols, dt in (("x16", s16, f16), ("x8", s8, f8))
                     if cols]
            xsets = []
            for s in range(2):
                streams = {}
                for key, cols, dt in sdefs:
                    streams[key] = [
                        wpool.tile([P, t1 - t0, N_CH, NB], dt,
                                   name=f"xp{s}_{key}_{pi}")
                        for pi, (t0, t1) in enumerate(_stream_pieces(cols))
                    ]
                xsets.append(streams)

            xt_fix = None
            if mode == "compute":
                # persistent x chunk for compute-only probe (chunk 0 data,
                # same per-col rhs dtypes as the real kernel)
                xt_fix = {}
                for key, cols, dt in sdefs:
                    xt_fix[key] = wpool.tile(
                        [P, len(cols), N_CH, NB], dt, name=f"xf_{key}")
                    nc.sync.dma_start(
                        out=xt_fix[key][:],
                        in_=x_exts[key][:, :len(cols) * N_CH * NB].rearrange(
                            "p (t c j) -> p t c j", c=N_CH, j=NB),
                    )
            if mode == "dma":
                # out is never written in the loop; write something once
                nc.scalar.dma_start(out=out_ext[0:P, :], in_=b3sb[:])

            import contextlib
            loop_cm = tc.For_i(0, n_iters, 1) if n_iters > 1 else contextlib.nullcontext()
            with loop_cm:
                for u in range(unroll):
                    _kernel_body(nc, tc, locals(), has_bias, mode,
                                 xt_fix, xsets[u % 2])

    nc.compile()
    return nc


def _kernel_body(nc, tc, env, has_bias, mode="full", xt_fix=None,
                 xset=None):
    x_exts = env["x_exts"]
    out_ext = env["out_ext"]
    s16, s8 = env["s16"], env["s8"]
    w1sb, w2sb, w3sb = env["w1sb"], env["w2sb"], env["w3sb"]
    b1sb, b2sb, b3sb = env["b1sb"], env["b2sb"], env["b3sb"]
    hp_pool, gt_pool, out_pool = env["hp_pool"], env["gt_pool"], env["out_pool"]
    ps1_pool, ps2_pool, ps3_pool = env["ps1_pool"], env["ps2_pool"], env["ps3_pool"]
    f16 = mybir.dt.float16
    f32 = mybir.dt.float32
    xqs = [getattr(nc, q) for q in XQ]
    # col -> (stream key, stream-col index)
    colmap = {}
    for key, cols in (("x16", s16), ("x8", s8)):
        for si, t in enumerate(cols):
            colmap[t] = (key, si)
    scols = {"x16": s16, "x8": s8}

    b0 = 0
    qi = 0
    for ch, nb in enumerate(CHUNKS):
        if mode == "compute":
            pieces = None
        else:
            # chunk ch of the pre-transposed stream-col-major x streams:
            # each piece is one fully contiguous DMA per partition
            pieces = {}
            for key, tiles in xset.items():
                cols = scols[key]
                c0 = len(cols) * N_CH * b0
                src = x_exts[key][:, c0:c0 + len(cols) * N_CH * nb].rearrange(
                    "p (t c j) -> p t c j", c=N_CH, j=nb
                )
                plist = []
                for pi, (t0, t1) in enumerate(_stream_pieces(cols)):
                    xp = tiles[pi][:, :t1 - t0, :, :nb]
                    xqs[qi % len(xqs)].dma_start(
                        out=xp[:], in_=src[:, t0:t1, :, :])
                    qi += 1
                    plist.append(xp)
                pieces[key] = plist
        if mode == "dma":
            b0 += nb
            continue

        def xt_rhs(c, t):
            key, si = colmap[t]
            if pieces is None:
                return xt_fix[key][:, si, c, :nb]
            pi, tl = si // PIECE_COLS, si % PIECE_COLS
            return pieces[key][pi][:, tl, c, :]

        hps = {}
        gt_t = gt_pool.tile([P, 2, NB], f16, name="gtt")
        gt = gt_t[:, :, :nb]
        for m in range(N_PAIR):
            # ---- layer 1, pair m: banded matmuls, t-major so the pair
            # starts as soon as its first tile-column lands ----
            ps_t = ps1_pool.tile([P, NB], f32, name="ps1t")
            ps = ps_t[:, :nb]
            if ALIGN_PAIRS:
                mm_list = [(c, tau) for tau in range(N_ATILE)
                           for c in range(N_CH)]
                cols_of = lambda tau: N_ATILE * m + tau
            else:
                mm_list = [(c, t) for t in _pair_tiles(m) for c in range(N_CH)]
                cols_of = lambda t: t
            for i, (c, t) in enumerate(mm_list):
                nc.tensor.matmul(
                    ps[:],
                    w1sb[:, BLK_IDX[(m, c, t)], :],
                    xt_rhs(c, cols_of(t)),
                    start=(i == 0),
                    stop=(i == len(mm_list) - 1),
                )
            hp_t = hp_pool.tile([P, NB], f16, name="hpt")
            hp = hp_t[:, :nb]
            # alternate DVE / Act for the PSUM->SBUF h copies (GPSIMD/Pool
            # cannot read PSUM)
            if has_bias:
                if m % 2 == 0:
                    nc.vector.tensor_scalar_add(hp[:], ps[:], b1sb[:, m:m + 1])
                else:
                    nc.scalar.add(hp[:], ps[:], b1sb[:, m:m + 1])
            else:
                if m % 2 == 0:
                    nc.vector.tensor_copy(out=hp[:], in_=ps[:])
                else:
                    nc.scalar.copy(out=hp[:], in_=ps[:])
            hps[m] = hp

            # ---- layer 2, group g right after its two pairs ----
            if m % 2 == 1:
                g = m // 2
                ps2_t = ps2_pool.tile([64, NB], f32, name="ps2t")
                ps2 = ps2_t[:, :nb]
                for piece in range(2):
                    nc.tensor.matmul(
                        ps2[:],
                        w2sb[:, 2 * g + piece, :],
                        hps[2 * g + piece][:],
                        start=(piece == 0),
                        stop=(piece == 1),
                    )
                lo = 64 * (g % 2)
                if has_bias:
                    nc.vector.tensor_scalar_add(
                        gt[lo:lo + 64, g // 2], ps2[:], b2sb[:, g:g + 1],
                    )
                else:
                    nc.vector.tensor_copy(out=gt[lo:lo + 64, g // 2], in_=ps2[:])

        # ---- layer 3: back to batch-major ----
        nj = nb // P
        osb_t = out_pool.tile([P, NB // P, N_OUT], f32, name="osbt")
        osb = osb_t[:, :nj]
        for js in range(nj):
            ps3 = ps3_pool.tile([P, N_OUT], f32)
            for piece in range(2):
                nc.tensor.matmul(
                    ps3[:],
                    gt[:, piece, js * P:(js + 1) * P],
                    w3sb[:, piece, :],
                    start=(piece == 0),
                    stop=(piece == 1),
                )
            # spread the 4 osb copies over Act/DVE
            if has_bias:
                if js % 2 == 0:
                    nc.scalar.add(osb[:, js], ps3[:], b3sb[:])
                else:
                    nc.vector.tensor_tensor(
                        osb[:, js], ps3[:], b3sb[:], mybir.AluOpType.add,
                    )
            else:
                if js % 2 == 0:
                    nc.scalar.copy(out=osb[:, js], in_=ps3[:])
                else:
                    nc.vector.tensor_copy(out=osb[:, js], in_=ps3[:])
        nc.scalar.dma_start(
            out=out_ext[b0:b0 + nb, :].rearrange("(j p) n -> p j n", p=P),
            in_=osb[:],
        )
        b0 += nb


_CACHED_NC = None

# Store W1 as fp8 e3m4 (scaled by W1_SCALE into e3m4's dense range).
# fp16 LDWEIGHTS measured ~53ns serialized per matmul (compute-only probe
# 45.5us fp16-W1 vs 42.9us fp8-W1 on more matmuls); fp8 W1 makes LDW
# ~free, worth ~8.5us/body at 176 matmuls.
W1_F8 = True
W1_SCALE = 64.0


def _prep_in_maps(x, W1, b1, W2, b2, W3, b3):
    xf = np.asarray(x, dtype=np.float32)
    w1sb, w2sb, w3sb, b1t, b2t, b3t = _pack_weights(W1, b1, W2, b2, W3, b3)
    in_maps = []
    for i in range(N_CORES):
        m = {
            "w1": w1sb,
            "w2": w2sb,
            "w3": w3sb,
            "b1t": b1t,
            "b2t": b2t,
            "b3t": b3t,
        }
        m.update(_pack_x_streams(xf[i * B_SHARD:(i + 1) * B_SHARD]))
        in_maps.append(m)
    return in_maps


_CACHED_BIAS_NC = None


def kernel(x, W1, b1, W2, b2, W3, b3):
    global _CACHED_NC, _CACHED_BIAS_NC
    has_bias = bool(
        np.any(np.asarray(b1)) or np.any(np.asarray(b2)) or np.any(np.asarray(b3))
    )
    if has_bias:
        if _CACHED_BIAS_NC is None:
            _CACHED_BIAS_NC = build_kernel(has_bias=True)
        nc = _CACHED_BIAS_NC
    else:
        if _CACHED_NC is None:
            _CACHED_NC = build_kernel()
        nc = _CACHED_NC
    in_maps = _prep_in_maps(x, W1, b1, W2, b2, W3, b3)
    last_err = None
    for attempt in range(3):
        try:
            res = run_bass_kernel_spmd(nc, in_maps, core_ids=list(range(N_CORES)))
            break
        except Exception as e:  # transient device/axon failures
            last_err = e
            if attempt == 2:
                raise
            import time as _time
            _time.sleep(20.0)
    return np.concatenate([res.results[i]["out"] for i in range(N_CORES)], axis=0)
